# revision 2
# baseline (speedup 1.0000x reference)
"""GQA kernel for Trainium2, 8 NeuronCores.

Problem: x[1,4096,2048], H=16 heads, G=4 kv-groups, D=128, causal mask,
RoPE on q/k, out-proj. Sharding: 2 heads per core (core c -> heads 2c,2c+1,
kv-group c//2). All tensors are fed to the device pre-transposed so every
matmul contraction dim lands on SBUF partitions:

  phase 1: QT/KT/V projections from xT (streamed once) + RoPE
  phase 2: per-head causal attention in scoresT orientation:
           scoresT[k,q] tiles -> exp (ACT, scale=1/sqrt(D)) -> causal mask by
           affine_select -> ctxT accum on PE; softmax denom = ones-matmul over
           a DVE-accumulated exp-sum tile; normalize via PE broadcast matmul.
  phase 3: AllGather ctxT (4MB/core) then column-parallel out-proj.

Output per core: outT_c = out.T[c*256:(c+1)*256, :]; host concatenates and
transposes back.
"""

import sys

for _p in ("/opt/trn_rl_repo",):
    if _p not in sys.path:
        sys.path.append(_p)

from contextlib import ExitStack

import numpy as np

import concourse.bass as bass
import concourse.tile as tile
from concourse import masks, mybir
from concourse.bass_utils import run_bass_kernel_spmd

F32 = mybir.dt.float32
S = 4096
MAX_WAITS = 1  # walrus CoreV3 rejects instructions with more sync waits


def _split_sync_waits(nc, maxw=MAX_WAITS):
    """Hoist excess sem waits onto NOPs inserted before the instruction on
    the same engine queue (queue order makes this equivalent)."""
    from concourse import mybir as mb
    n = 0
    for bassbb in nc.bb_map.values():
        bb = bassbb.bb
        insts = list(bb.instructions)
        out = []
        changed = False
        for ins in insts:
            si = ins.sync_info
            if si is not None and si.on_wait and len(si.on_wait) > maxw:
                waits = list(si.on_wait)
                head, rest = waits[:-maxw], waits[-maxw:]
                while head:
                    chunk, head = head[:maxw], head[maxw:]
                    n += 1
                    nop = mb.InstNoOp(
                        name=f"I-ws{n}",
                        engine=ins.engine,
                        ins=[],
                        outs=[],
                        sync_info=mb.SyncInfo(on_wait=chunk, on_update=[]),
                    )
                    nc.register_instruction(nop)
                    out.append(nop)
                ins.sync_info = mb.SyncInfo(
                    on_wait=rest, on_update=list(si.on_update or []))
                changed = True
            out.append(ins)
        if changed:
            try:
                bb.instructions[:] = out
            except TypeError:
                bb.set_instructions(out)
    return n
DIN = 2048
D = 128
HPC = 2          # heads per core
NCORES = 8
QC = 512         # q-chunk (free dim per matmul)
NQ = S // QC     # 8 q-chunks
KT = 128         # k tile (partition dim)
NKIN = DIN // 128  # 16 contraction tiles for projections
INV_SQRT_D = 1.0 / np.sqrt(D)


def build_nc(debug=False):
    nc = bass.Bass(num_devices=NCORES)

    xT = nc.dram_tensor("xT", [DIN, S], F32, kind="ExternalInput")
    wqT = nc.dram_tensor("wqT", [DIN, HPC * D], F32, kind="ExternalInput")
    wkT = nc.dram_tensor("wkT", [DIN, D], F32, kind="ExternalInput")
    wvT = nc.dram_tensor("wvT", [DIN, D], F32, kind="ExternalInput")
    cosT = nc.dram_tensor("cosT", [D, S], F32, kind="ExternalInput")
    sinT = nc.dram_tensor("sinT", [D, S], F32, kind="ExternalInput")
    woT = nc.dram_tensor("woT", [DIN, HPC * D], F32, kind="ExternalInput")
    outT = nc.dram_tensor("outT", [HPC * D, S], F32, kind="ExternalOutput")

    # collective bounce buffers (collectives can't touch I/O tensors;
    # input must NOT be Shared, output should be Shared)
    ctx_local = nc.dram_tensor("ctx_local", [HPC * D, S], F32)
    ctx_full = nc.dram_tensor("ctx_full", [NCORES * HPC * D, S], F32,
                              addr_space="Shared")
    if debug:
        dbg_qt = nc.dram_tensor("dbg_qt", [128, S], F32, kind="ExternalOutput")
        dbg_kt = nc.dram_tensor("dbg_kt", [128, S], F32, kind="ExternalOutput")
        dbg_vt = nc.dram_tensor("dbg_vt", [128, S // 128, D], F32,
                                kind="ExternalOutput")
        dbg_cl = nc.dram_tensor("dbg_cl", [HPC * D, S], F32,
                                kind="ExternalOutput")
        dbg_cf = nc.dram_tensor("dbg_cf", [NCORES * HPC * D, S], F32,
                                kind="ExternalOutput")

    with ExitStack() as ctx:
        tc = ctx.enter_context(tile.TileContext(nc))

        res = ctx.enter_context(tc.tile_pool(name="res", bufs=1))
        # resident SBUF tensors
        qt = res.tile([128, HPC, S], F32, tag="qt")          # QT per head
        kt = res.tile([128, S], F32, tag="kt")               # KT (shared group)
        vt = res.tile([128, S // 128, D], F32, tag="vt")     # V as s-tiles
        wq_sb = res.tile([128, NKIN, HPC * D], F32, tag="wq")
        wk_sb = res.tile([128, NKIN, D], F32, tag="wk")
        wv_sb = res.tile([128, NKIN, D], F32, tag="wv")
        wo_sb = res.tile([128, NKIN, HPC * D], F32, tag="wo")
        ones_k = res.tile([128, 1], F32, tag="ones_k")       # lhsT for col sums
        ones_r = res.tile([1, 128], F32, tag="ones_r")       # lhsT for bcast
        ident = res.tile([128, 128], F32, tag="ident")       # PE transpose id

        nc.vector.memset(ones_k, 1.0)
        nc.vector.memset(ones_r, 1.0)
        masks.make_identity(nc, ident)

        # weight loads
        nc.sync.dma_start(out=wq_sb, in_=wqT.rearrange("(t p) m -> p t m", p=128))
        nc.sync.dma_start(out=wk_sb, in_=wkT.rearrange("(t p) m -> p t m", p=128))
        nc.sync.dma_start(out=wv_sb, in_=wvT.rearrange("(t p) m -> p t m", p=128))
        nc.sync.dma_start(out=wo_sb, in_=woT.rearrange("(t p) m -> p t m", p=128))

        # ---------------- phase 1: projections + RoPE ----------------
        with ExitStack() as p1:
            p1_res = p1.enter_context(tc.tile_pool(name="p1res", bufs=1))
            cos_sb = p1_res.tile([128, S], F32, tag="cos")
            sin_sb = p1_res.tile([128, S], F32, tag="sin")
            nc.sync.dma_start(out=cos_sb, in_=cosT[:, :])
            nc.sync.dma_start(out=sin_sb, in_=sinT[:, :])

            xpool = p1.enter_context(tc.tile_pool(name="xpool", bufs=4))
            rpool = p1.enter_context(tc.tile_pool(name="rope", bufs=3))
            pq_pool = p1.enter_context(tc.tile_pool(name="pq", bufs=2, space="PSUM"))
            pk_pool = p1.enter_context(tc.tile_pool(name="pk", bufs=2, space="PSUM"))
            pvt_pool = p1.enter_context(tc.tile_pool(name="pvt", bufs=1, space="PSUM"))
            pv2_pool = p1.enter_context(tc.tile_pool(name="pv2", bufs=1, space="PSUM"))

            for qc in range(NQ):
                q0 = qc * QC
                pq = pq_pool.tile([128, HPC, QC], F32, tag="pq")
                pk = pk_pool.tile([128, QC], F32, tag="pk")
                pvt = pvt_pool.tile([128, QC], F32, tag="pvt")
                for ki in range(NKIN):
                    xt = xpool.tile([128, QC], F32, tag="xt")
                    nc.sync.dma_start(
                        out=xt, in_=xT[ki * 128:(ki + 1) * 128, q0:q0 + QC])
                    st = ki == 0
                    sp = ki == NKIN - 1
                    for h in range(HPC):
                        nc.tensor.matmul(
                            pq[:, h, :], lhsT=wq_sb[:, ki, h * D:(h + 1) * D],
                            rhs=xt, start=st, stop=sp)
                    nc.tensor.matmul(pk, lhsT=wk_sb[:, ki, :], rhs=xt,
                                     start=st, stop=sp)
                    nc.tensor.matmul(pvt, lhsT=wv_sb[:, ki, :], rhs=xt,
                                     start=st, stop=sp)
                # VT -> V via PE block transposes (each a single full write)
                vtT = rpool.tile([128, QC], F32, tag="vtT")
                nc.vector.tensor_copy(vtT, pvt)
                pv2 = pv2_pool.tile([128, 4, D], F32, tag="pv2")
                for si in range(4):
                    nc.tensor.transpose(
                        pv2[:, si, :], vtT[:, si * 128:(si + 1) * 128], ident)

                # RoPE: dest = src*cos + rot(src)*sin, rot along partitions
                cos_c = cos_sb[:, q0:q0 + QC]
                sin_c = sin_sb[:, q0:q0 + QC]
                for h in range(HPC):
                    src = pq[:, h, :]
                    dst = qt[:, h, q0:q0 + QC]
                    rot = rpool.tile([128, QC], F32, tag="rot")
                    nc.vector.tensor_scalar_mul(rot[0:64, :], src[64:128, :], -1.0)
                    nc.vector.tensor_copy(rot[64:128, :], src[0:64, :])
                    nc.vector.tensor_mul(dst, src, cos_c)
                    nc.vector.tensor_mul(rot, rot, sin_c)
                    nc.vector.tensor_add(dst, dst, rot)
                src = pk
                dst = kt[:, q0:q0 + QC]
                rot = rpool.tile([128, QC], F32, tag="rot")
                nc.vector.tensor_scalar_mul(rot[0:64, :], src[64:128, :], -1.0)
                nc.vector.tensor_copy(rot[64:128, :], src[0:64, :])
                nc.vector.tensor_mul(dst, src, cos_c)
                nc.vector.tensor_mul(rot, rot, sin_c)
                nc.vector.tensor_add(dst, dst, rot)

                nc.vector.tensor_copy(vt[:, qc * 4:(qc + 1) * 4, :], pv2)

        # ---------------- phase 2: attention ----------------
        with ExitStack() as p2:
            wpool = p2.enter_context(tc.tile_pool(name="wpool", bufs=4))
            apool = p2.enter_context(tc.tile_pool(name="acc", bufs=2))
            npool = p2.enter_context(tc.tile_pool(name="norm", bufs=2))
            copool = p2.enter_context(tc.tile_pool(name="cout", bufs=2))
            ps_pool = p2.enter_context(tc.tile_pool(name="ps", bufs=3, space="PSUM"))
            pc_pool = p2.enter_context(tc.tile_pool(name="pc", bufs=2, space="PSUM"))
            pe_pool = p2.enter_context(tc.tile_pool(name="pe", bufs=1, space="PSUM"))
            pb_pool = p2.enter_context(tc.tile_pool(name="pb", bufs=1, space="PSUM"))

            for h in range(HPC):
                for qc in range(NQ):
                    q0 = qc * QC
                    nk = (qc + 1) * 4
                    pc = pc_pool.tile([128, QC], F32, tag="pc")
                    acc = apool.tile([128, QC], F32, tag="acc")
                    for ki in range(nk):
                        k0 = ki * KT
                        ps = ps_pool.tile([128, QC], F32, tag="ps")
                        nc.tensor.matmul(ps, lhsT=kt[:, k0:k0 + KT],
                                         rhs=qt[:, h, q0:q0 + QC],
                                         start=True, stop=True)
                        wt = wpool.tile([128, QC], F32, tag="wt")
                        nc.scalar.activation(wt, ps,
                                             mybir.ActivationFunctionType.Exp,
                                             scale=INV_SQRT_D)
                        if k0 + KT - 1 > q0:
                            # keep where (q0+j) - (k0+p) >= 0
                            nc.gpsimd.affine_select(
                                out=wt, in_=wt, pattern=[[1, QC]],
                                compare_op=mybir.AluOpType.is_ge, fill=0.0,
                                base=q0 - k0, channel_multiplier=-1)
                        nc.tensor.matmul(pc, lhsT=vt[:, ki, :], rhs=wt,
                                         start=(ki == 0), stop=(ki == nk - 1))
                        if ki == 0:
                            nc.vector.tensor_copy(acc, wt)
                        else:
                            nc.vector.tensor_add(acc, acc, wt)
                    pe = pe_pool.tile([1, QC], F32, tag="pe")
                    nc.tensor.matmul(pe, lhsT=ones_k, rhs=acc,
                                     start=True, stop=True)
                    rec = npool.tile([1, QC], F32, tag="rec")
                    nc.vector.reciprocal(rec, pe)
                    pb = pb_pool.tile([128, QC], F32, tag="pb")
                    nc.tensor.matmul(pb, lhsT=ones_r, rhs=rec,
                                     start=True, stop=True)
                    bc = npool.tile([128, QC], F32, tag="bc")
                    nc.vector.tensor_copy(bc, pb)
                    cout = copool.tile([128, QC], F32, tag="cout")
                    nc.vector.tensor_mul(cout, pc, bc)
                    nc.sync.dma_start(
                        out=ctx_local[h * D:(h + 1) * D, q0:q0 + QC], in_=cout)

        if debug:
            nc.sync.dma_start(out=dbg_qt[:, :], in_=qt[:, 0, :])
            nc.sync.dma_start(out=dbg_kt[:, :], in_=kt)
            nc.sync.dma_start(out=dbg_vt[:, :, :], in_=vt)

        # ---------------- allgather ----------------
        tc.strict_bb_all_engine_barrier()
        nc.gpsimd.collective_compute(
            "AllGather",
            mybir.AluOpType.bypass,
            replica_groups=[list(range(NCORES))],
            ins=[ctx_local[:, :]],
            outs=[ctx_full[:, :]],
        )
        tc.strict_bb_all_engine_barrier()
        if debug:
            nc.sync.dma_start(out=dbg_cl[:, :], in_=ctx_local[:, :])
            nc.sync.dma_start(out=dbg_cf[:, :], in_=ctx_full[:, :])

        # ---------------- phase 3: out-proj ----------------
        with ExitStack() as p3:
            cpool = p3.enter_context(tc.tile_pool(name="cpool", bufs=4))
            opool = p3.enter_context(tc.tile_pool(name="opool", bufs=2))
            po_pool = p3.enter_context(tc.tile_pool(name="po", bufs=2, space="PSUM"))
            for sc in range(NQ):
                s0 = sc * QC
                po = po_pool.tile([128, HPC, QC], F32, tag="po")
                for ti in range(NKIN):
                    ct = cpool.tile([128, QC], F32, tag="ct")
                    nc.sync.dma_start(
                        out=ct, in_=ctx_full[ti * 128:(ti + 1) * 128, s0:s0 + QC])
                    for m in range(HPC):
                        nc.tensor.matmul(
                            po[:, m, :], lhsT=wo_sb[:, ti, m * D:(m + 1) * D],
                            rhs=ct, start=(ti == 0), stop=(ti == NKIN - 1))
                ot = opool.tile([128, HPC, QC], F32, tag="ot")
                nc.vector.tensor_copy(ot, po)
                for m in range(HPC):
                    nc.sync.dma_start(
                        out=outT[m * 128:(m + 1) * 128, s0:s0 + QC],
                        in_=ot[:, m, :])

    _split_sync_waits(nc)
    return nc


_NC_CACHE = None


def _get_nc():
    global _NC_CACHE
    if _NC_CACHE is None:
        _NC_CACHE = build_nc()
    return _NC_CACHE


def _make_in_maps(x, cos, sin, Wq, Wk, Wv, Wo):
    xT = np.ascontiguousarray(x.reshape(S, DIN).T)
    cosT = np.ascontiguousarray(cos.T)
    sinT = np.ascontiguousarray(sin.T)
    in_maps = []
    for c in range(NCORES):
        g = c // 2
        in_maps.append({
            "xT": xT,
            "wqT": np.ascontiguousarray(Wq[c * 256:(c + 1) * 256, :].T),
            "wkT": np.ascontiguousarray(Wk[g * 128:(g + 1) * 128, :].T),
            "wvT": np.ascontiguousarray(Wv[g * 128:(g + 1) * 128, :].T),
            "cosT": cosT,
            "sinT": sinT,
            "woT": np.ascontiguousarray(Wo[c * 256:(c + 1) * 256, :].T),
        })
    return in_maps


def run(x, cos, sin, Wq, Wk, Wv, Wo, trace=False, tmpdir=None):
    nc = _get_nc()
    in_maps = _make_in_maps(x, cos, sin, Wq, Wk, Wv, Wo)
    res = run_bass_kernel_spmd(nc, in_maps, list(range(NCORES)), trace=trace,
                               tmpdir=tmpdir)
    outT = np.concatenate([res.results[c]["outT"] for c in range(NCORES)], axis=0)
    out = np.ascontiguousarray(outT.T).reshape(1, S, DIN).astype(np.float32)
    return out, res


def kernel(x, mask, cos, sin, Wq, Wk, Wv, Wo):
    out, _ = run(np.asarray(x, dtype=np.float32), np.asarray(cos, np.float32),
                 np.asarray(sin, np.float32), np.asarray(Wq, np.float32),
                 np.asarray(Wk, np.float32), np.asarray(Wv, np.float32),
                 np.asarray(Wo, np.float32))
    return out



# revision 16
# speedup vs baseline: 3.1108x; 3.1108x over previous
"""GQA kernel for Trainium2, 8 NeuronCores.

Problem: x[1,4096,2048], H=16 heads, G=4 kv-groups, D=128, causal mask,
RoPE on q/k, out-proj. Sharding: 2 heads per core (core c -> heads 2c,2c+1,
kv-group c//2).

v2 design (vs fp32 baseline):
  - fp16 data path everywhere (PE runs 1 cycle/row vs 4 for fp32; DMA and
    SBUF halved). PSUM accumulation stays fp32; softmax denominator reduced
    in fp32 by gpsimd partition_all_reduce.
  - fused projection+attention loop over q-chunks of 512 so PE/ACT/DVE/DMA
    overlap across phases.
  - softmax: exp(s/sqrt(D) - 2) on ACT (bias cancels in normalization),
    causal mask via gpsimd affine_select on diagonal tiles, denominator by
    DVE fp16 adds + partition_all_reduce, normalize by DVE mul with
    reciprocal tile.
  - out-proj sequence-sharded: AllToAll exchanges per-head ctx columns
    (2MB/core instead of 32MB AllGather); each core computes the full
    2048-dim output for its 512 sequence positions with the whole Wo
    (8MB fp16, preloaded during attention).
"""

import sys

for _p in ("/opt/trn_rl_repo",):
    if _p not in sys.path:
        sys.path.append(_p)

from contextlib import ExitStack

import numpy as np

import concourse.bass as bass
import concourse.bass_isa as bass_isa
import concourse.tile as tile
from concourse import masks, mybir
from concourse.bass_utils import run_bass_kernel_spmd

F32 = mybir.dt.float32
F16 = mybir.dt.float16
S = 4096
MAX_WAITS = 1  # walrus CoreV3 rejects instructions with more sync waits


def _split_sync_waits(nc, maxw=MAX_WAITS):
    """Hoist excess sem waits onto NOPs inserted before the instruction on
    the same engine queue (queue order makes this equivalent)."""
    from concourse import mybir as mb
    n = 0
    for bassbb in nc.bb_map.values():
        bb = bassbb.bb
        insts = list(bb.instructions)
        out = []
        changed = False
        for ins in insts:
            si = ins.sync_info
            if si is not None and si.on_wait and len(si.on_wait) > maxw:
                waits = list(si.on_wait)
                head, rest = waits[:-maxw], waits[-maxw:]
                while head:
                    chunk, head = head[:maxw], head[maxw:]
                    n += 1
                    nop = mb.InstNoOp(
                        name=f"I-ws{n}",
                        engine=ins.engine,
                        ins=[],
                        outs=[],
                        sync_info=mb.SyncInfo(on_wait=chunk, on_update=[]),
                    )
                    nc.register_instruction(nop)
                    out.append(nop)
                ins.sync_info = mb.SyncInfo(
                    on_wait=rest, on_update=list(si.on_update or []))
                changed = True
            out.append(ins)
        if changed:
            try:
                bb.instructions[:] = out
            except TypeError:
                bb.set_instructions(out)
    return n


DIN = 2048
D = 128
HPC = 2          # heads per core
NCORES = 8
QC = 512         # q-chunk (free dim per matmul)
NQ = S // QC     # 8 q-chunks
KT = 128         # k tile (partition dim)
NKIN = DIN // 128  # 16 contraction tiles for projections
INV_SQRT_D = 1.0 / np.sqrt(D)
EXP_BIAS = -2.0  # keeps fp16 softmax sums well inside range; cancels in norm


def build_nc():
    nc = bass.Bass(num_devices=NCORES)

    xT = nc.dram_tensor("xT", [DIN, S], F16, kind="ExternalInput")
    wqT = nc.dram_tensor("wqT", [DIN, HPC * D], F16, kind="ExternalInput")
    wkT = nc.dram_tensor("wkT", [DIN, D], F16, kind="ExternalInput")
    wvT = nc.dram_tensor("wvT", [DIN, D], F16, kind="ExternalInput")
    cosT = nc.dram_tensor("cosT", [D, S], F16, kind="ExternalInput")
    sinT = nc.dram_tensor("sinT", [D, S], F16, kind="ExternalInput")
    woT = nc.dram_tensor("woT", [DIN, DIN], F16, kind="ExternalInput")
    outT = nc.dram_tensor("outT", [DIN, QC], F32, kind="ExternalOutput")

    # collective bounce buffers (collectives can't touch I/O tensors;
    # input must NOT be Shared, output should be Shared)
    cc_in = nc.dram_tensor("cc_in", [NCORES, HPC * D, QC], F16)
    cc_out = nc.dram_tensor("cc_out", [NCORES, HPC * D, QC], F16)

    with ExitStack() as ctx:
        tc = ctx.enter_context(tile.TileContext(nc))

        res = ctx.enter_context(tc.tile_pool(name="res", bufs=1))
        # resident SBUF tensors
        qt = res.tile([128, HPC, S], F16, tag="qt")          # QT per head
        kt = res.tile([128, S], F16, tag="kt")               # KT (shared group)
        vt = res.tile([128, S // 128, D], F16, tag="vt")     # V as s-tiles
        wq_sb = res.tile([128, NKIN, HPC * D], F16, tag="wq")
        wk_sb = res.tile([128, NKIN, D], F16, tag="wk")
        wv_sb = res.tile([128, NKIN, D], F16, tag="wv")
        wo_sb = res.tile([128, NKIN, DIN], F16, tag="wo")
        cos_sb = res.tile([128, S], F16, tag="cos")
        sin_sb = res.tile([128, S], F16, tag="sin")
        ident = res.tile([128, 128], F32, tag="ident")       # PE transpose id
        ebias = res.tile([128, 1], F32, tag="ebias")         # exp bias const
        ones128 = res.tile([128, 128], F16, tag="ones128")   # partition reduce+bcast

        masks.make_identity(nc, ident)
        nc.vector.memset(ebias, EXP_BIAS)
        nc.vector.memset(ones128, 1.0)

        # weight loads (wo is big; issued last so the small ones land first)
        nc.sync.dma_start(out=wq_sb, in_=wqT.rearrange("(t p) m -> p t m", p=128))
        nc.sync.dma_start(out=wk_sb, in_=wkT.rearrange("(t p) m -> p t m", p=128))
        nc.sync.dma_start(out=wv_sb, in_=wvT.rearrange("(t p) m -> p t m", p=128))
        nc.sync.dma_start(out=cos_sb, in_=cosT[:, :])
        nc.sync.dma_start(out=sin_sb, in_=sinT[:, :])
        nc.sync.dma_start(out=wo_sb, in_=woT.rearrange("(t p) m -> p t m", p=128))

        # ---------------- fused projections + RoPE + attention ----------------
        with ExitStack() as p2:
            xpool = p2.enter_context(tc.tile_pool(name="xpool", bufs=2))
            rpool = p2.enter_context(tc.tile_pool(name="rope", bufs=3))
            wpool = p2.enter_context(tc.tile_pool(name="wpool", bufs=4))
            apool = p2.enter_context(tc.tile_pool(name="acc", bufs=2))
            npool = p2.enter_context(tc.tile_pool(name="norm", bufs=2))
            copool = p2.enter_context(tc.tile_pool(name="cout", bufs=2))
            # PSUM: 8 banks of [128, 2KB]: pq=2, pk=1, pvt=1, ps=2, pv2=1
            # (in ps_pool with bufs=1), pc=1, pd=1
            pq_pool = p2.enter_context(tc.tile_pool(name="pq", bufs=1, space="PSUM"))
            pk_pool = p2.enter_context(tc.tile_pool(name="pk", bufs=1, space="PSUM"))
            pvt_pool = p2.enter_context(tc.tile_pool(name="pvt", bufs=1, space="PSUM"))
            ps_pool = p2.enter_context(tc.tile_pool(name="ps", bufs=2, space="PSUM"))
            pc_pool = p2.enter_context(tc.tile_pool(name="pc", bufs=1, space="PSUM"))
            pd_pool = p2.enter_context(tc.tile_pool(name="pd", bufs=1, space="PSUM"))

            for qc in range(NQ):
                q0 = qc * QC
                # -------- projections --------
                # Q heads projected sequentially into a single PSUM bank;
                # K/V interleaved in their own banks.
                xt = xpool.tile([128, NKIN, QC], F16, tag="xt")
                nc.sync.dma_start(
                    out=xt,
                    in_=xT.rearrange("(t p) m -> p t m", p=128)[:, :, q0:q0 + QC])
                q2 = rpool.tile([128, HPC, QC], F16, tag="q2")
                for h in range(HPC):
                    pq = pq_pool.tile([128, QC], F32, tag="pq")
                    for ki in range(NKIN):
                        nc.tensor.matmul(
                            pq, lhsT=wq_sb[:, ki, h * D:(h + 1) * D],
                            rhs=xt[:, ki, :], start=(ki == 0),
                            stop=(ki == NKIN - 1))
                    nc.vector.tensor_copy(q2[:, h, :], pq)
                pk = pk_pool.tile([128, QC], F32, tag="pk")
                pvt = pvt_pool.tile([128, QC], F32, tag="pvt")
                for ki in range(NKIN):
                    st = ki == 0
                    sp = ki == NKIN - 1
                    nc.tensor.matmul(pk, lhsT=wk_sb[:, ki, :], rhs=xt[:, ki, :],
                                     start=st, stop=sp)
                    nc.tensor.matmul(pvt, lhsT=wv_sb[:, ki, :], rhs=xt[:, ki, :],
                                     start=st, stop=sp)
                # VT -> V via PE block transposes (fp32: transpose dtype must
                # match input; converted to fp16 on the copy out)
                vtT = rpool.tile([128, QC], F32, tag="vtT")
                nc.vector.tensor_copy(vtT, pvt)
                pv2 = ps_pool.tile([128, 4, D], F32, tag="pv2", bufs=1)
                for si in range(4):
                    nc.tensor.transpose(
                        pv2[:, si, :], vtT[:, si * 128:(si + 1) * 128], ident)
                nc.vector.tensor_copy(vt[:, qc * 4:(qc + 1) * 4, :], pv2)

                # -------- RoPE (all-fp16 on DVE) --------
                cos_c = cos_sb[:, q0:q0 + QC]
                sin_c = sin_sb[:, q0:q0 + QC]
                k1 = rpool.tile([128, QC], F16, tag="k1")
                nc.vector.tensor_copy(k1, pk)

                def rope(dst, src):
                    rot = rpool.tile([128, QC], F16, tag="rot")
                    nc.vector.tensor_scalar_mul(rot[0:64, :], src[64:128, :], -1.0)
                    nc.vector.tensor_copy(rot[64:128, :], src[0:64, :])
                    nc.vector.tensor_mul(dst, src, cos_c)
                    nc.vector.tensor_mul(rot, rot, sin_c)
                    nc.vector.tensor_add(dst, dst, rot)

                for h in range(HPC):
                    rope(qt[:, h, q0:q0 + QC], q2[:, h, :])
                rope(kt[:, q0:q0 + QC], k1)

                # -------- attention for this q-chunk --------
                nk = (qc + 1) * 4
                for h in range(HPC):
                    pc = pc_pool.tile([128, QC], F32, tag="pc")
                    acc = apool.tile([128, QC], F16, tag="acc")
                    for ki in range(nk):
                        k0 = ki * KT
                        ps = ps_pool.tile([128, QC], F32, tag="ps")
                        nc.tensor.matmul(ps, lhsT=kt[:, k0:k0 + KT],
                                         rhs=qt[:, h, q0:q0 + QC],
                                         start=True, stop=True)
                        wt = wpool.tile([128, QC], F16, tag="wt")
                        nc.scalar.activation(wt, ps,
                                             mybir.ActivationFunctionType.Exp,
                                             scale=INV_SQRT_D, bias=ebias)
                        if k0 + KT - 1 > q0:
                            # keep where (q0+j) - (k0+p) >= 0
                            nc.gpsimd.affine_select(
                                out=wt, in_=wt, pattern=[[1, QC]],
                                compare_op=mybir.AluOpType.is_ge, fill=0.0,
                                base=q0 - k0, channel_multiplier=-1)
                        nc.tensor.matmul(pc, lhsT=vt[:, ki, :], rhs=wt,
                                         start=(ki == 0), stop=(ki == nk - 1))
                        if ki == 0:
                            nc.vector.tensor_copy(acc, wt)
                        else:
                            nc.vector.tensor_add(acc, acc, wt)
                    # denominator: all-ones matmul reduces over partitions AND
                    # broadcasts the sum to every partition in one instruction
                    pd = pd_pool.tile([128, QC], F32, tag="pd")
                    nc.tensor.matmul(pd, lhsT=ones128, rhs=acc,
                                     start=True, stop=True)
                    rec = npool.tile([128, QC], F32, tag="rec")
                    nc.vector.reciprocal(rec, pd)
                    cout = copool.tile([128, QC], F16, tag="cout")
                    nc.vector.tensor_mul(cout, pc, rec)
                    nc.sync.dma_start(
                        out=cc_in[qc, h * D:(h + 1) * D, :], in_=cout)

        # ---------------- all-to-all (seq-shard the context) ----------------
        tc.strict_bb_all_engine_barrier()
        nc.gpsimd.collective_compute(
            "AllToAll",
            mybir.AluOpType.bypass,
            replica_groups=[list(range(NCORES))],
            ins=[cc_in[:, :, :]],
            outs=[cc_out[:, :, :]],
        )
        tc.strict_bb_all_engine_barrier()

        # ---------------- out-proj for my 512 columns ----------------
        with ExitStack() as p3:
            cpool = p3.enter_context(tc.tile_pool(name="cpool", bufs=1))
            opool = p3.enter_context(tc.tile_pool(name="opool", bufs=2))
            po_pool = p3.enter_context(tc.tile_pool(name="po", bufs=2, space="PSUM"))
            cc_sb = cpool.tile([128, NKIN, QC], F16, tag="cc_sb")
            nc.sync.dma_start(
                out=cc_sb,
                in_=cc_out.rearrange("c (t p) m -> p (c t) m", p=128))
            for dt in range(NKIN):
                po = po_pool.tile([128, QC], F32, tag="po")
                for ki in range(NKIN):
                    nc.tensor.matmul(
                        po, lhsT=wo_sb[:, ki, dt * 128:(dt + 1) * 128],
                        rhs=cc_sb[:, ki, :], start=(ki == 0), stop=(ki == NKIN - 1))
                ot = opool.tile([128, QC], F32, tag="ot")
                nc.vector.tensor_copy(ot, po)
                nc.sync.dma_start(
                    out=outT[dt * 128:(dt + 1) * 128, :], in_=ot)

    _split_sync_waits(nc)
    return nc


_NC_CACHE = None


def _get_nc():
    global _NC_CACHE
    if _NC_CACHE is None:
        _NC_CACHE = build_nc()
    return _NC_CACHE


def _make_in_maps(x, cos, sin, Wq, Wk, Wv, Wo):
    xT = np.ascontiguousarray(x.reshape(S, DIN).T.astype(np.float16))
    cosT = np.ascontiguousarray(cos.T.astype(np.float16))
    sinT = np.ascontiguousarray(sin.T.astype(np.float16))
    woT = np.ascontiguousarray(Wo.T.astype(np.float16))
    in_maps = []
    for c in range(NCORES):
        g = c // 2
        in_maps.append({
            "xT": xT,
            "wqT": np.ascontiguousarray(Wq[c * 256:(c + 1) * 256, :].T
                                        .astype(np.float16)),
            "wkT": np.ascontiguousarray(Wk[g * 128:(g + 1) * 128, :].T
                                        .astype(np.float16)),
            "wvT": np.ascontiguousarray(Wv[g * 128:(g + 1) * 128, :].T
                                        .astype(np.float16)),
            "cosT": cosT,
            "sinT": sinT,
            "woT": woT,
        })
    return in_maps


def run(x, cos, sin, Wq, Wk, Wv, Wo, trace=False, tmpdir=None):
    nc = _get_nc()
    in_maps = _make_in_maps(x, cos, sin, Wq, Wk, Wv, Wo)
    res = run_bass_kernel_spmd(nc, in_maps, list(range(NCORES)), trace=trace,
                               tmpdir=tmpdir)
    out = np.empty((1, S, DIN), dtype=np.float32)
    for c in range(NCORES):
        out[0, c * QC:(c + 1) * QC, :] = res.results[c]["outT"].T
    return out, res


def kernel(x, mask, cos, sin, Wq, Wk, Wv, Wo):
    out, _ = run(np.asarray(x, dtype=np.float32), np.asarray(cos, np.float32),
                 np.asarray(sin, np.float32), np.asarray(Wq, np.float32),
                 np.asarray(Wk, np.float32), np.asarray(Wv, np.float32),
                 np.asarray(Wo, np.float32))
    return out


# revision 21
# speedup vs baseline: 3.4636x; 1.1134x over previous
"""GQA kernel for Trainium2, 8 NeuronCores.

Problem: x[1,4096,2048], H=16 heads, G=4 kv-groups, D=128, causal mask,
RoPE on q/k, out-proj. Sharding: 2 heads per core (core c -> heads 2c,2c+1,
kv-group c//2).

v2 design (vs fp32 baseline):
  - fp16 data path everywhere (PE runs 1 cycle/row vs 4 for fp32; DMA and
    SBUF halved). PSUM accumulation stays fp32; softmax denominator reduced
    in fp32 by gpsimd partition_all_reduce.
  - fused projection+attention loop over q-chunks of 512 so PE/ACT/DVE/DMA
    overlap across phases.
  - softmax: exp(s/sqrt(D) - 2) on ACT (bias cancels in normalization),
    causal mask via gpsimd affine_select on diagonal tiles, denominator by
    DVE fp16 adds + partition_all_reduce, normalize by DVE mul with
    reciprocal tile.
  - out-proj sequence-sharded: AllToAll exchanges per-head ctx columns
    (2MB/core instead of 32MB AllGather); each core computes the full
    2048-dim output for its 512 sequence positions with the whole Wo
    (8MB fp16, preloaded during attention).
"""

import sys

for _p in ("/opt/trn_rl_repo",):
    if _p not in sys.path:
        sys.path.append(_p)

from contextlib import ExitStack

import numpy as np

import concourse.bass as bass
import concourse.bass_isa as bass_isa
import concourse.tile as tile
from concourse import masks, mybir
from concourse.bass_utils import run_bass_kernel_spmd

F32 = mybir.dt.float32
F16 = mybir.dt.float16
S = 4096
MAX_WAITS = 1  # walrus CoreV3 rejects instructions with more sync waits


def _split_sync_waits(nc, maxw=MAX_WAITS):
    """Hoist excess sem waits onto NOPs inserted before the instruction on
    the same engine queue (queue order makes this equivalent)."""
    from concourse import mybir as mb
    n = 0
    for bassbb in nc.bb_map.values():
        bb = bassbb.bb
        insts = list(bb.instructions)
        out = []
        changed = False
        for ins in insts:
            si = ins.sync_info
            if si is not None and si.on_wait and len(si.on_wait) > maxw:
                waits = list(si.on_wait)
                head, rest = waits[:-maxw], waits[-maxw:]
                while head:
                    chunk, head = head[:maxw], head[maxw:]
                    n += 1
                    nop = mb.InstNoOp(
                        name=f"I-ws{n}",
                        engine=ins.engine,
                        ins=[],
                        outs=[],
                        sync_info=mb.SyncInfo(on_wait=chunk, on_update=[]),
                    )
                    nc.register_instruction(nop)
                    out.append(nop)
                ins.sync_info = mb.SyncInfo(
                    on_wait=rest, on_update=list(si.on_update or []))
                changed = True
            out.append(ins)
        if changed:
            try:
                bb.instructions[:] = out
            except TypeError:
                bb.set_instructions(out)
    return n


DIN = 2048
D = 128
HPC = 2          # heads per core
NCORES = 8
QC = 512         # q-chunk (free dim per matmul)
NQ = S // QC     # 8 q-chunks
KT = 128         # k tile (partition dim)
NKIN = DIN // 128  # 16 contraction tiles for projections
INV_SQRT_D = 1.0 / np.sqrt(D)
EXP_BIAS = -2.0  # keeps fp16 softmax sums well inside range; cancels in norm


def build_nc():
    nc = bass.Bass(num_devices=NCORES)

    xT = nc.dram_tensor("xT", [DIN, S], F16, kind="ExternalInput")
    wqT = nc.dram_tensor("wqT", [DIN, HPC * D], F16, kind="ExternalInput")
    wkT = nc.dram_tensor("wkT", [DIN, D], F16, kind="ExternalInput")
    wvT = nc.dram_tensor("wvT", [DIN, D], F16, kind="ExternalInput")
    cosT = nc.dram_tensor("cosT", [D, S], F16, kind="ExternalInput")
    sinT = nc.dram_tensor("sinT", [D, S], F16, kind="ExternalInput")
    woT = nc.dram_tensor("woT", [DIN, DIN], F16, kind="ExternalInput")
    outT = nc.dram_tensor("outT", [DIN, QC], F32, kind="ExternalOutput")

    # collective bounce buffers (collectives can't touch I/O tensors;
    # input must NOT be Shared, output should be Shared)
    cc_in = nc.dram_tensor("cc_in", [NCORES, HPC * D, QC], F16)
    cc_out = nc.dram_tensor("cc_out", [NCORES, HPC * D, QC], F16)

    with ExitStack() as ctx:
        tc = ctx.enter_context(tile.TileContext(nc))

        res = ctx.enter_context(tc.tile_pool(name="res", bufs=1))
        # resident SBUF tensors
        qt = res.tile([128, HPC, S], F16, tag="qt")          # QT per head
        kt = res.tile([128, S], F16, tag="kt")               # KT (shared group)
        vt = res.tile([128, S // 128, D], F16, tag="vt")     # V as s-tiles
        wq_sb = res.tile([128, NKIN, HPC * D], F16, tag="wq")
        wk_sb = res.tile([128, NKIN, D], F16, tag="wk")
        wv_sb = res.tile([128, NKIN, D], F16, tag="wv")
        wo_sb = res.tile([128, NKIN, DIN], F16, tag="wo")
        cos_sb = res.tile([128, S], F16, tag="cos")
        sin_sb = res.tile([128, S], F16, tag="sin")
        ebias = res.tile([128, 1], F32, tag="ebias")         # exp bias const
        ones128 = res.tile([128, 128], F16, tag="ones128")   # partition reduce+bcast

        nc.vector.memset(ebias, EXP_BIAS)
        nc.vector.memset(ones128, 1.0)

        # weight loads (wo is big; issued last so the small ones land first)
        nc.sync.dma_start(out=wq_sb, in_=wqT.rearrange("(t p) m -> p t m", p=128))
        nc.sync.dma_start(out=wk_sb, in_=wkT.rearrange("(t p) m -> p t m", p=128))
        nc.sync.dma_start(out=wv_sb, in_=wvT.rearrange("(t p) m -> p t m", p=128))
        nc.sync.dma_start(out=cos_sb, in_=cosT[:, :])
        nc.sync.dma_start(out=sin_sb, in_=sinT[:, :])
        nc.sync.dma_start(out=wo_sb, in_=woT.rearrange("(t p) m -> p t m", p=128))

        # ---------------- fused projections + RoPE + attention ----------------
        with ExitStack() as p2:
            xpool = p2.enter_context(tc.tile_pool(name="xpool", bufs=2))
            rpool = p2.enter_context(tc.tile_pool(name="rope", bufs=3))
            wpool = p2.enter_context(tc.tile_pool(name="wpool", bufs=4))
            apool = p2.enter_context(tc.tile_pool(name="acc", bufs=2))
            npool = p2.enter_context(tc.tile_pool(name="norm", bufs=2))
            copool = p2.enter_context(tc.tile_pool(name="cout", bufs=2))
            # PSUM: 8 banks of [128, 2KB]: pq=1, pk=1, pvt=1, ps=3, pc=1, pd=1
            pq_pool = p2.enter_context(tc.tile_pool(name="pq", bufs=1, space="PSUM"))
            pk_pool = p2.enter_context(tc.tile_pool(name="pk", bufs=1, space="PSUM"))
            pvt_pool = p2.enter_context(tc.tile_pool(name="pvt", bufs=1, space="PSUM"))
            ps_pool = p2.enter_context(tc.tile_pool(name="ps", bufs=3, space="PSUM"))
            pc_pool = p2.enter_context(tc.tile_pool(name="pc", bufs=1, space="PSUM"))
            pd_pool = p2.enter_context(tc.tile_pool(name="pd", bufs=1, space="PSUM"))

            for qc in range(NQ):
                q0 = qc * QC
                # -------- projections --------
                # Q heads projected sequentially into a single PSUM bank;
                # K/V interleaved in their own banks.
                xt = xpool.tile([128, NKIN, QC], F16, tag="xt")
                nc.sync.dma_start(
                    out=xt,
                    in_=xT.rearrange("(t p) m -> p t m", p=128)[:, :, q0:q0 + QC])
                q2 = rpool.tile([128, HPC, QC], F16, tag="q2")
                for h in range(HPC):
                    pq = pq_pool.tile([128, QC], F32, tag="pq")
                    for ki in range(NKIN):
                        nc.tensor.matmul(
                            pq, lhsT=wq_sb[:, ki, h * D:(h + 1) * D],
                            rhs=xt[:, ki, :], start=(ki == 0),
                            stop=(ki == NKIN - 1))
                    nc.vector.tensor_copy(q2[:, h, :], pq)
                pk = pk_pool.tile([128, QC], F32, tag="pk")
                pvt = pvt_pool.tile([128, QC], F32, tag="pvt")
                for ki in range(NKIN):
                    st = ki == 0
                    sp = ki == NKIN - 1
                    nc.tensor.matmul(pk, lhsT=wk_sb[:, ki, :], rhs=xt[:, ki, :],
                                     start=st, stop=sp)
                    nc.tensor.matmul(pvt, lhsT=wv_sb[:, ki, :], rhs=xt[:, ki, :],
                                     start=st, stop=sp)
                # VT -> V via DMA XBAR transpose (fp16, runs on DMA engines:
                # out[p, m, d] = in[d, m*128 + p], exactly vt's layout)
                vtT = rpool.tile([128, QC], F16, tag="vtT")
                nc.vector.tensor_copy(vtT, pvt)
                nc.sync.dma_start_transpose(
                    out=vt[:, qc * 4:(qc + 1) * 4, :], in_=vtT)

                # -------- RoPE (all-fp16 on DVE) --------
                cos_c = cos_sb[:, q0:q0 + QC]
                sin_c = sin_sb[:, q0:q0 + QC]
                k1 = rpool.tile([128, QC], F16, tag="k1")
                nc.vector.tensor_copy(k1, pk)

                def rope(dst, src):
                    rot = rpool.tile([128, QC], F16, tag="rot")
                    nc.vector.tensor_scalar_mul(rot[0:64, :], src[64:128, :], -1.0)
                    nc.vector.tensor_copy(rot[64:128, :], src[0:64, :])
                    nc.vector.tensor_mul(dst, src, cos_c)
                    nc.vector.tensor_mul(rot, rot, sin_c)
                    nc.vector.tensor_add(dst, dst, rot)

                for h in range(HPC):
                    rope(qt[:, h, q0:q0 + QC], q2[:, h, :])
                rope(kt[:, q0:q0 + QC], k1)

                # -------- attention for this q-chunk --------
                # software-pipelined: scores run LAG tiles ahead of the PV
                # matmuls so the PE never waits on the ACT exp latency
                nk = (qc + 1) * 4
                LAG = 2
                for h in range(HPC):
                    pc = pc_pool.tile([128, QC], F32, tag="pc")
                    acc = apool.tile([128, QC], F16, tag="acc")
                    wts = {}

                    def emit_scores(ki):
                        k0 = ki * KT
                        ps = ps_pool.tile([128, QC], F32, tag="ps")
                        nc.tensor.matmul(ps, lhsT=kt[:, k0:k0 + KT],
                                         rhs=qt[:, h, q0:q0 + QC],
                                         start=True, stop=True)
                        wt = wpool.tile([128, QC], F16, tag="wt")
                        nc.scalar.activation(wt, ps,
                                             mybir.ActivationFunctionType.Exp,
                                             scale=INV_SQRT_D, bias=ebias)
                        if k0 + KT - 1 > q0:
                            # keep where (q0+j) - (k0+p) >= 0
                            nc.gpsimd.affine_select(
                                out=wt, in_=wt, pattern=[[1, QC]],
                                compare_op=mybir.AluOpType.is_ge, fill=0.0,
                                base=q0 - k0, channel_multiplier=-1)
                        wts[ki] = wt

                    def emit_pv(ki):
                        wt = wts.pop(ki)
                        nc.tensor.matmul(pc, lhsT=vt[:, ki, :], rhs=wt,
                                         start=(ki == 0), stop=(ki == nk - 1))
                        if ki == 0:
                            nc.vector.tensor_copy(acc, wt)
                        else:
                            nc.vector.tensor_add(acc, acc, wt)

                    for ki in range(nk):
                        emit_scores(ki)
                        if ki >= LAG:
                            emit_pv(ki - LAG)
                    for ki in range(nk - LAG, nk):
                        emit_pv(ki)
                    # denominator: all-ones matmul reduces over partitions AND
                    # broadcasts the sum to every partition in one instruction
                    pd = pd_pool.tile([128, QC], F32, tag="pd")
                    nc.tensor.matmul(pd, lhsT=ones128, rhs=acc,
                                     start=True, stop=True)
                    # 1/d as exp(-ln(d)) on ACT (DVE reciprocal costs 3.4us
                    # per tile; custom-DVE approx ops don't compile here)
                    lg = npool.tile([128, QC], F32, tag="lg")
                    nc.scalar.activation(lg, pd,
                                         mybir.ActivationFunctionType.Ln,
                                         scale=1.0)
                    rec = npool.tile([128, QC], F16, tag="rec")
                    nc.scalar.activation(rec, lg,
                                         mybir.ActivationFunctionType.Exp,
                                         scale=-1.0)
                    cout = copool.tile([128, QC], F16, tag="cout")
                    nc.vector.tensor_mul(cout, pc, rec)
                    nc.sync.dma_start(
                        out=cc_in[qc, h * D:(h + 1) * D, :], in_=cout)

        # ---------------- all-to-all (seq-shard the context) ----------------
        tc.strict_bb_all_engine_barrier()
        nc.gpsimd.collective_compute(
            "AllToAll",
            mybir.AluOpType.bypass,
            replica_groups=[list(range(NCORES))],
            ins=[cc_in[:, :, :]],
            outs=[cc_out[:, :, :]],
        )
        tc.strict_bb_all_engine_barrier()

        # ---------------- out-proj for my 512 columns ----------------
        with ExitStack() as p3:
            cpool = p3.enter_context(tc.tile_pool(name="cpool", bufs=1))
            opool = p3.enter_context(tc.tile_pool(name="opool", bufs=2))
            po_pool = p3.enter_context(tc.tile_pool(name="po", bufs=2, space="PSUM"))
            cc_sb = cpool.tile([128, NKIN, QC], F16, tag="cc_sb")
            nc.sync.dma_start(
                out=cc_sb,
                in_=cc_out.rearrange("c (t p) m -> p (c t) m", p=128))
            for dt in range(NKIN):
                po = po_pool.tile([128, QC], F32, tag="po")
                for ki in range(NKIN):
                    nc.tensor.matmul(
                        po, lhsT=wo_sb[:, ki, dt * 128:(dt + 1) * 128],
                        rhs=cc_sb[:, ki, :], start=(ki == 0), stop=(ki == NKIN - 1))
                ot = opool.tile([128, QC], F32, tag="ot")
                nc.vector.tensor_copy(ot, po)
                nc.sync.dma_start(
                    out=outT[dt * 128:(dt + 1) * 128, :], in_=ot)

    _split_sync_waits(nc)
    return nc


_NC_CACHE = None


def _get_nc():
    global _NC_CACHE
    if _NC_CACHE is None:
        _NC_CACHE = build_nc()
    return _NC_CACHE


def _make_in_maps(x, cos, sin, Wq, Wk, Wv, Wo):
    xT = np.ascontiguousarray(x.reshape(S, DIN).T.astype(np.float16))
    cosT = np.ascontiguousarray(cos.T.astype(np.float16))
    sinT = np.ascontiguousarray(sin.T.astype(np.float16))
    woT = np.ascontiguousarray(Wo.T.astype(np.float16))
    in_maps = []
    for c in range(NCORES):
        g = c // 2
        in_maps.append({
            "xT": xT,
            "wqT": np.ascontiguousarray(Wq[c * 256:(c + 1) * 256, :].T
                                        .astype(np.float16)),
            "wkT": np.ascontiguousarray(Wk[g * 128:(g + 1) * 128, :].T
                                        .astype(np.float16)),
            "wvT": np.ascontiguousarray(Wv[g * 128:(g + 1) * 128, :].T
                                        .astype(np.float16)),
            "cosT": cosT,
            "sinT": sinT,
            "woT": woT,
        })
    return in_maps


def run(x, cos, sin, Wq, Wk, Wv, Wo, trace=False, tmpdir=None):
    nc = _get_nc()
    in_maps = _make_in_maps(x, cos, sin, Wq, Wk, Wv, Wo)
    res = run_bass_kernel_spmd(nc, in_maps, list(range(NCORES)), trace=trace,
                               tmpdir=tmpdir)
    out = np.empty((1, S, DIN), dtype=np.float32)
    for c in range(NCORES):
        out[0, c * QC:(c + 1) * QC, :] = res.results[c]["outT"].T
    return out, res


def kernel(x, mask, cos, sin, Wq, Wk, Wv, Wo):
    out, _ = run(np.asarray(x, dtype=np.float32), np.asarray(cos, np.float32),
                 np.asarray(sin, np.float32), np.asarray(Wq, np.float32),
                 np.asarray(Wk, np.float32), np.asarray(Wv, np.float32),
                 np.asarray(Wo, np.float32))
    return out


# revision 27
# speedup vs baseline: 3.4672x; 1.0011x over previous
"""GQA kernel for Trainium2, 8 NeuronCores.

Problem: x[1,4096,2048], H=16 heads, G=4 kv-groups, D=128, causal mask,
RoPE on q/k, out-proj. Sharding: 2 heads per core (core c -> heads 2c,2c+1,
kv-group c//2).

v2 design (vs fp32 baseline):
  - fp16 data path everywhere (PE runs 1 cycle/row vs 4 for fp32; DMA and
    SBUF halved). PSUM accumulation stays fp32; softmax denominator reduced
    in fp32 by gpsimd partition_all_reduce.
  - fused projection+attention loop over q-chunks of 512 so PE/ACT/DVE/DMA
    overlap across phases.
  - softmax: exp(s/sqrt(D) - 2) on ACT (bias cancels in normalization),
    causal mask via gpsimd affine_select on diagonal tiles, denominator by
    DVE fp16 adds + partition_all_reduce, normalize by DVE mul with
    reciprocal tile.
  - out-proj sequence-sharded: AllToAll exchanges per-head ctx columns
    (2MB/core instead of 32MB AllGather); each core computes the full
    2048-dim output for its 512 sequence positions with the whole Wo
    (8MB fp16, preloaded during attention).
"""

import sys

for _p in ("/opt/trn_rl_repo",):
    if _p not in sys.path:
        sys.path.append(_p)

from contextlib import ExitStack

import numpy as np

import concourse.bass as bass
import concourse.bass_isa as bass_isa
import concourse.tile as tile
from concourse import masks, mybir
from concourse.bass_utils import run_bass_kernel_spmd

F32 = mybir.dt.float32
F16 = mybir.dt.float16
S = 4096
MAX_WAITS = 1  # walrus CoreV3 rejects instructions with more sync waits


def _split_sync_waits(nc, maxw=MAX_WAITS):
    """Hoist excess sem waits onto NOPs inserted before the instruction on
    the same engine queue (queue order makes this equivalent)."""
    from concourse import mybir as mb
    n = 0
    for bassbb in nc.bb_map.values():
        bb = bassbb.bb
        insts = list(bb.instructions)
        out = []
        changed = False
        for ins in insts:
            si = ins.sync_info
            if si is not None and si.on_wait and len(si.on_wait) > maxw:
                waits = list(si.on_wait)
                head, rest = waits[:-maxw], waits[-maxw:]
                while head:
                    chunk, head = head[:maxw], head[maxw:]
                    n += 1
                    nop = mb.InstNoOp(
                        name=f"I-ws{n}",
                        engine=ins.engine,
                        ins=[],
                        outs=[],
                        sync_info=mb.SyncInfo(on_wait=chunk, on_update=[]),
                    )
                    nc.register_instruction(nop)
                    out.append(nop)
                ins.sync_info = mb.SyncInfo(
                    on_wait=rest, on_update=list(si.on_update or []))
                changed = True
            out.append(ins)
        if changed:
            try:
                bb.instructions[:] = out
            except TypeError:
                bb.set_instructions(out)
    return n


DIN = 2048
D = 128
HPC = 2          # heads per core
NCORES = 8
QC = 512         # q-chunk (free dim per matmul)
NQ = S // QC     # 8 q-chunks
KT = 128         # k tile (partition dim)
NKIN = DIN // 128  # 16 contraction tiles for projections
INV_SQRT_D = 1.0 / np.sqrt(D)
EXP_BIAS = -2.0  # keeps fp16 softmax sums well inside range; cancels in norm


def build_nc():
    nc = bass.Bass(num_devices=NCORES)

    xT = nc.dram_tensor("xT", [DIN, S], F16, kind="ExternalInput")
    wqT = nc.dram_tensor("wqT", [DIN, HPC * D], F16, kind="ExternalInput")
    wkT = nc.dram_tensor("wkT", [DIN, D], F16, kind="ExternalInput")
    wvT = nc.dram_tensor("wvT", [DIN, D], F16, kind="ExternalInput")
    cosT = nc.dram_tensor("cosT", [D, S], F16, kind="ExternalInput")
    sinT = nc.dram_tensor("sinT", [D, S], F16, kind="ExternalInput")
    woT = nc.dram_tensor("woT", [DIN, DIN], F16, kind="ExternalInput")
    outT = nc.dram_tensor("outT", [DIN, QC], F32, kind="ExternalOutput")

    # collective bounce buffers (collectives can't touch I/O tensors;
    # input must NOT be Shared, output should be Shared)
    cc_in = nc.dram_tensor("cc_in", [NCORES, HPC * D, QC], F16)
    cc_out = nc.dram_tensor("cc_out", [NCORES, HPC * D, QC], F16)

    with ExitStack() as ctx:
        tc = ctx.enter_context(tile.TileContext(nc))

        res = ctx.enter_context(tc.tile_pool(name="res", bufs=1))
        # resident SBUF tensors
        qt = res.tile([128, HPC, S], F16, tag="qt")          # QT per head
        kt = res.tile([128, S], F16, tag="kt")               # KT (shared group)
        vt = res.tile([128, S // 128, D], F16, tag="vt")     # V as s-tiles
        wq_sb = res.tile([128, NKIN, HPC * D], F16, tag="wq")
        wk_sb = res.tile([128, NKIN, D], F16, tag="wk")
        wv_sb = res.tile([128, NKIN, D], F16, tag="wv")
        wo_sb = res.tile([128, NKIN, DIN], F16, tag="wo")
        cos_sb = res.tile([128, S], F16, tag="cos")
        sin_sb = res.tile([128, S], F16, tag="sin")
        ebias = res.tile([128, 1], F32, tag="ebias")         # exp bias const
        ones128 = res.tile([128, 128], F16, tag="ones128")   # partition reduce+bcast

        nc.vector.memset(ebias, EXP_BIAS)
        nc.vector.memset(ones128, 1.0)

        # weight loads; the big wo (8MB, needed only in phase 3) goes on the
        # scalar engine's DMA queue so it streams in the background without
        # delaying the x-chunk loads on the sync queue
        nc.sync.dma_start(out=wq_sb, in_=wqT.rearrange("(t p) m -> p t m", p=128))
        nc.sync.dma_start(out=wk_sb, in_=wkT.rearrange("(t p) m -> p t m", p=128))
        nc.sync.dma_start(out=wv_sb, in_=wvT.rearrange("(t p) m -> p t m", p=128))
        nc.sync.dma_start(out=cos_sb, in_=cosT[:, :])
        nc.sync.dma_start(out=sin_sb, in_=sinT[:, :])
        nc.scalar.dma_start(out=wo_sb, in_=woT.rearrange("(t p) m -> p t m", p=128))

        # ---------------- fused projections + RoPE + attention ----------------
        with ExitStack() as p2:
            xpool = p2.enter_context(tc.tile_pool(name="xpool", bufs=2))
            rpool = p2.enter_context(tc.tile_pool(name="rope", bufs=3))
            wpool = p2.enter_context(tc.tile_pool(name="wpool", bufs=4))
            apool = p2.enter_context(tc.tile_pool(name="acc", bufs=2))
            npool = p2.enter_context(tc.tile_pool(name="norm", bufs=2))
            copool = p2.enter_context(tc.tile_pool(name="cout", bufs=2))
            # PSUM: 8 banks of [128, 2KB]: pq=1, pk=1, pvt=1, ps=2+pd=1, pc=2
            pq_pool = p2.enter_context(tc.tile_pool(name="pq", bufs=1, space="PSUM"))
            pk_pool = p2.enter_context(tc.tile_pool(name="pk", bufs=1, space="PSUM"))
            pvt_pool = p2.enter_context(tc.tile_pool(name="pvt", bufs=1, space="PSUM"))
            ps_pool = p2.enter_context(tc.tile_pool(name="ps", bufs=2, space="PSUM"))
            pc_pool = p2.enter_context(tc.tile_pool(name="pc", bufs=2, space="PSUM"))

            def load_xt(qc):
                t = xpool.tile([128, NKIN, QC], F16, tag="xt")
                nc.sync.dma_start(
                    out=t,
                    in_=xT.rearrange("(t p) m -> p t m", p=128)
                          [:, :, qc * QC:(qc + 1) * QC])
                return t

            xt_next = load_xt(0)
            for qc in range(NQ):
                q0 = qc * QC
                # -------- projections --------
                # Q heads projected sequentially into a single PSUM bank;
                # K/V interleaved in their own banks.
                xt = xt_next
                q2 = rpool.tile([128, HPC, QC], F16, tag="q2")
                for h in range(HPC):
                    pq = pq_pool.tile([128, QC], F32, tag="pq")
                    for ki in range(NKIN):
                        nc.tensor.matmul(
                            pq, lhsT=wq_sb[:, ki, h * D:(h + 1) * D],
                            rhs=xt[:, ki, :], start=(ki == 0),
                            stop=(ki == NKIN - 1))
                    nc.vector.tensor_copy(q2[:, h, :], pq)
                pk = pk_pool.tile([128, QC], F32, tag="pk")
                pvt = pvt_pool.tile([128, QC], F32, tag="pvt")
                for ki in range(NKIN):
                    st = ki == 0
                    sp = ki == NKIN - 1
                    nc.tensor.matmul(pk, lhsT=wk_sb[:, ki, :], rhs=xt[:, ki, :],
                                     start=st, stop=sp)
                    nc.tensor.matmul(pvt, lhsT=wv_sb[:, ki, :], rhs=xt[:, ki, :],
                                     start=st, stop=sp)
                # prefetch next x-chunk while this chunk's attention runs
                if qc + 1 < NQ:
                    xt_next = load_xt(qc + 1)
                # VT -> V via DMA XBAR transpose (fp16, runs on DMA engines:
                # out[p, m, d] = in[d, m*128 + p], exactly vt's layout)
                vtT = rpool.tile([128, QC], F16, tag="vtT")
                nc.vector.tensor_copy(vtT, pvt)
                nc.sync.dma_start_transpose(
                    out=vt[:, qc * 4:(qc + 1) * 4, :], in_=vtT)

                # -------- RoPE (all-fp16 on DVE) --------
                cos_c = cos_sb[:, q0:q0 + QC]
                sin_c = sin_sb[:, q0:q0 + QC]
                k1 = rpool.tile([128, QC], F16, tag="k1")
                nc.vector.tensor_copy(k1, pk)

                def rope(dst, src):
                    rot = rpool.tile([128, QC], F16, tag="rot")
                    nc.vector.tensor_scalar_mul(rot[0:64, :], src[64:128, :], -1.0)
                    nc.vector.tensor_copy(rot[64:128, :], src[0:64, :])
                    nc.vector.tensor_mul(dst, src, cos_c)
                    nc.vector.tensor_mul(rot, rot, sin_c)
                    nc.vector.tensor_add(dst, dst, rot)

                for h in range(HPC):
                    rope(qt[:, h, q0:q0 + QC], q2[:, h, :])
                rope(kt[:, q0:q0 + QC], k1)

                # -------- attention for this q-chunk --------
                # both heads share one software-pipelined stream: scores run
                # one k-tile ahead of the PV matmuls so the PE never waits on
                # the ACT exp latency
                nk = (qc + 1) * 4
                LAG = 1
                pcs = [pc_pool.tile([128, QC], F32, tag="pc", name=f"pc{h}")
                       for h in range(HPC)]
                accs = [apool.tile([128, QC], F16, tag="acc", name=f"acc{h}")
                        for h in range(HPC)]
                wts = {}

                def emit_scores(h, ki):
                    k0 = ki * KT
                    ps = ps_pool.tile([128, QC], F32, tag="ps")
                    nc.tensor.matmul(ps, lhsT=kt[:, k0:k0 + KT],
                                     rhs=qt[:, h, q0:q0 + QC],
                                     start=True, stop=True)
                    wt = wpool.tile([128, QC], F16, tag="wt")
                    nc.scalar.activation(wt, ps,
                                         mybir.ActivationFunctionType.Exp,
                                         scale=INV_SQRT_D, bias=ebias)
                    if k0 + KT - 1 > q0:
                        # keep where (q0+j) - (k0+p) >= 0
                        nc.gpsimd.affine_select(
                            out=wt, in_=wt, pattern=[[1, QC]],
                            compare_op=mybir.AluOpType.is_ge, fill=0.0,
                            base=q0 - k0, channel_multiplier=-1)
                    wts[(h, ki)] = wt

                def emit_pv(h, ki):
                    wt = wts.pop((h, ki))
                    nc.tensor.matmul(pcs[h], lhsT=vt[:, ki, :], rhs=wt,
                                     start=(ki == 0), stop=(ki == nk - 1))
                    if ki == 0:
                        nc.vector.tensor_copy(accs[h], wt)
                    else:
                        nc.vector.tensor_add(accs[h], accs[h], wt)

                for ki in range(nk):
                    for h in range(HPC):
                        emit_scores(h, ki)
                    if ki >= LAG:
                        for h in range(HPC):
                            emit_pv(h, ki - LAG)
                for ki in range(nk - LAG, nk):
                    for h in range(HPC):
                        emit_pv(h, ki)
                for h in range(HPC):
                    # denominator: all-ones matmul reduces over partitions AND
                    # broadcasts the sum to every partition in one instruction
                    pd = ps_pool.tile([128, QC], F32, tag="pd", bufs=1)
                    nc.tensor.matmul(pd, lhsT=ones128, rhs=accs[h],
                                     start=True, stop=True)
                    # 1/d as exp(-ln(d)) on ACT (DVE reciprocal costs 3.4us
                    # per tile; custom-DVE approx ops don't compile here)
                    lg = npool.tile([128, QC], F32, tag="lg")
                    nc.scalar.activation(lg, pd,
                                         mybir.ActivationFunctionType.Ln,
                                         scale=1.0)
                    rec = npool.tile([128, QC], F16, tag="rec")
                    nc.scalar.activation(rec, lg,
                                         mybir.ActivationFunctionType.Exp,
                                         scale=-1.0)
                    cout = copool.tile([128, QC], F16, tag="cout")
                    nc.vector.tensor_mul(cout, pcs[h], rec)
                    nc.gpsimd.dma_start(
                        out=cc_in[qc, h * D:(h + 1) * D, :], in_=cout)

        # ---------------- all-to-all (seq-shard the context) ----------------
        tc.strict_bb_all_engine_barrier()
        nc.gpsimd.collective_compute(
            "AllToAll",
            mybir.AluOpType.bypass,
            replica_groups=[list(range(NCORES))],
            ins=[cc_in[:, :, :]],
            outs=[cc_out[:, :, :]],
        )
        tc.strict_bb_all_engine_barrier()

        # ---------------- out-proj for my 512 columns ----------------
        with ExitStack() as p3:
            cpool = p3.enter_context(tc.tile_pool(name="cpool", bufs=1))
            opool = p3.enter_context(tc.tile_pool(name="opool", bufs=2))
            po_pool = p3.enter_context(tc.tile_pool(name="po", bufs=2, space="PSUM"))
            cc_sb = cpool.tile([128, NKIN, QC], F16, tag="cc_sb")
            nc.sync.dma_start(
                out=cc_sb,
                in_=cc_out.rearrange("c (t p) m -> p (c t) m", p=128))
            for dt in range(NKIN):
                po = po_pool.tile([128, QC], F32, tag="po")
                for ki in range(NKIN):
                    nc.tensor.matmul(
                        po, lhsT=wo_sb[:, ki, dt * 128:(dt + 1) * 128],
                        rhs=cc_sb[:, ki, :], start=(ki == 0), stop=(ki == NKIN - 1))
                ot = opool.tile([128, QC], F32, tag="ot")
                nc.vector.tensor_copy(ot, po)
                nc.sync.dma_start(
                    out=outT[dt * 128:(dt + 1) * 128, :], in_=ot)

    _split_sync_waits(nc)
    return nc


_NC_CACHE = None


def _get_nc():
    global _NC_CACHE
    if _NC_CACHE is None:
        _NC_CACHE = build_nc()
    return _NC_CACHE


def _make_in_maps(x, cos, sin, Wq, Wk, Wv, Wo):
    xT = np.ascontiguousarray(x.reshape(S, DIN).T.astype(np.float16))
    cosT = np.ascontiguousarray(cos.T.astype(np.float16))
    sinT = np.ascontiguousarray(sin.T.astype(np.float16))
    woT = np.ascontiguousarray(Wo.T.astype(np.float16))
    in_maps = []
    for c in range(NCORES):
        g = c // 2
        in_maps.append({
            "xT": xT,
            "wqT": np.ascontiguousarray(Wq[c * 256:(c + 1) * 256, :].T
                                        .astype(np.float16)),
            "wkT": np.ascontiguousarray(Wk[g * 128:(g + 1) * 128, :].T
                                        .astype(np.float16)),
            "wvT": np.ascontiguousarray(Wv[g * 128:(g + 1) * 128, :].T
                                        .astype(np.float16)),
            "cosT": cosT,
            "sinT": sinT,
            "woT": woT,
        })
    return in_maps


def run(x, cos, sin, Wq, Wk, Wv, Wo, trace=False, tmpdir=None):
    nc = _get_nc()
    in_maps = _make_in_maps(x, cos, sin, Wq, Wk, Wv, Wo)
    res = run_bass_kernel_spmd(nc, in_maps, list(range(NCORES)), trace=trace,
                               tmpdir=tmpdir)
    out = np.empty((1, S, DIN), dtype=np.float32)
    for c in range(NCORES):
        out[0, c * QC:(c + 1) * QC, :] = res.results[c]["outT"].T
    return out, res


def kernel(x, mask, cos, sin, Wq, Wk, Wv, Wo):
    out, _ = run(np.asarray(x, dtype=np.float32), np.asarray(cos, np.float32),
                 np.asarray(sin, np.float32), np.asarray(Wq, np.float32),
                 np.asarray(Wk, np.float32), np.asarray(Wv, np.float32),
                 np.asarray(Wo, np.float32))
    return out


# revision 33
# speedup vs baseline: 3.6597x; 1.0555x over previous
"""GQA kernel for Trainium2, 8 NeuronCores.

Problem: x[1,4096,2048], H=16 heads, G=4 kv-groups, D=128, causal mask,
RoPE on q/k, out-proj. Sharding: 2 heads per core (core c -> heads 2c,2c+1,
kv-group c//2).

v2 design (vs fp32 baseline):
  - fp16 data path everywhere (PE runs 1 cycle/row vs 4 for fp32; DMA and
    SBUF halved). PSUM accumulation stays fp32; softmax denominator reduced
    in fp32 by gpsimd partition_all_reduce.
  - fused projection+attention loop over q-chunks of 512 so PE/ACT/DVE/DMA
    overlap across phases.
  - softmax: exp(s/sqrt(D) - 2) on ACT (bias cancels in normalization),
    causal mask via gpsimd affine_select on diagonal tiles, denominator by
    DVE fp16 adds + partition_all_reduce, normalize by DVE mul with
    reciprocal tile.
  - out-proj sequence-sharded: AllToAll exchanges per-head ctx columns
    (2MB/core instead of 32MB AllGather); each core computes the full
    2048-dim output for its 512 sequence positions with the whole Wo
    (8MB fp16, preloaded during attention).
"""

import sys

for _p in ("/opt/trn_rl_repo",):
    if _p not in sys.path:
        sys.path.append(_p)

from contextlib import ExitStack

import numpy as np

import concourse.bass as bass
import concourse.bass_isa as bass_isa
import concourse.tile as tile
from concourse import masks, mybir
from concourse.bass_utils import run_bass_kernel_spmd

F32 = mybir.dt.float32
F16 = mybir.dt.float16
S = 4096
MAX_WAITS = 1  # walrus CoreV3 rejects instructions with more sync waits


def _split_sync_waits(nc, maxw=MAX_WAITS):
    """Hoist excess sem waits onto NOPs inserted before the instruction on
    the same engine queue (queue order makes this equivalent)."""
    from concourse import mybir as mb
    n = 0
    for bassbb in nc.bb_map.values():
        bb = bassbb.bb
        insts = list(bb.instructions)
        out = []
        changed = False
        for ins in insts:
            si = ins.sync_info
            if si is not None and si.on_wait and len(si.on_wait) > maxw:
                waits = list(si.on_wait)
                head, rest = waits[:-maxw], waits[-maxw:]
                while head:
                    chunk, head = head[:maxw], head[maxw:]
                    n += 1
                    nop = mb.InstNoOp(
                        name=f"I-ws{n}",
                        engine=ins.engine,
                        ins=[],
                        outs=[],
                        sync_info=mb.SyncInfo(on_wait=chunk, on_update=[]),
                    )
                    nc.register_instruction(nop)
                    out.append(nop)
                ins.sync_info = mb.SyncInfo(
                    on_wait=rest, on_update=list(si.on_update or []))
                changed = True
            out.append(ins)
        if changed:
            try:
                bb.instructions[:] = out
            except TypeError:
                bb.set_instructions(out)
    return n


DIN = 2048
D = 128
HPC = 2          # heads per core
NCORES = 8
QC = 512         # q-chunk (free dim per matmul)
NQ = S // QC     # 8 q-chunks
KT = 128         # k tile (partition dim)
NKIN = DIN // 128  # 16 contraction tiles for projections
INV_SQRT_D = 1.0 / np.sqrt(D)
EXP_BIAS = -2.0  # keeps fp16 softmax sums well inside range; cancels in norm


def build_nc():
    nc = bass.Bass(num_devices=NCORES)

    xT = nc.dram_tensor("xT", [DIN, S], F16, kind="ExternalInput")
    wqT = nc.dram_tensor("wqT", [DIN, HPC * D], F16, kind="ExternalInput")
    wkT = nc.dram_tensor("wkT", [DIN, D], F16, kind="ExternalInput")
    wvT = nc.dram_tensor("wvT", [DIN, D], F16, kind="ExternalInput")
    cosT = nc.dram_tensor("cosT", [D, S], F16, kind="ExternalInput")
    sinT = nc.dram_tensor("sinT", [D, S], F16, kind="ExternalInput")
    woT = nc.dram_tensor("woT", [DIN, DIN], F16, kind="ExternalInput")
    outT = nc.dram_tensor("outT", [DIN, QC], F32, kind="ExternalOutput")

    # collective bounce buffers (collectives can't touch I/O tensors;
    # input must NOT be Shared, output should be Shared)
    # exchange buffers split into column halves so the second AllToAll
    # overlaps the first half's out-projection
    CH = QC // 2
    cc_inA = nc.dram_tensor("cc_inA", [NCORES, HPC * D, CH], F16)
    cc_inB = nc.dram_tensor("cc_inB", [NCORES, HPC * D, CH], F16)
    cc_outA = nc.dram_tensor("cc_outA", [NCORES, HPC * D, CH], F16)
    cc_outB = nc.dram_tensor("cc_outB", [NCORES, HPC * D, CH], F16)

    with ExitStack() as ctx:
        tc = ctx.enter_context(tile.TileContext(nc))

        res = ctx.enter_context(tc.tile_pool(name="res", bufs=1))
        # resident SBUF tensors
        qt = res.tile([128, HPC, S], F16, tag="qt")          # QT per head
        kt = res.tile([128, S], F16, tag="kt")               # KT (shared group)
        vt = res.tile([128, S // 128, D], F16, tag="vt")     # V as s-tiles
        wq_sb = res.tile([128, NKIN, HPC * D], F16, tag="wq")
        wk_sb = res.tile([128, NKIN, D], F16, tag="wk")
        wv_sb = res.tile([128, NKIN, D], F16, tag="wv")
        wo_sb = res.tile([128, NKIN, DIN], F16, tag="wo")
        cos_sb = res.tile([128, S], F16, tag="cos")
        sin_sb = res.tile([128, S], F16, tag="sin")
        ebias = res.tile([128, 1], F32, tag="ebias")         # exp bias const
        ones128 = res.tile([128, 128], F16, tag="ones128")   # partition reduce+bcast

        nc.vector.memset(ebias, EXP_BIAS)
        nc.vector.memset(ones128, 1.0)

        # weight loads: wq/wk/wv needed at the first matmul, cos/sin at the
        # first RoPE; the big wo (8MB, phase 3 only) is deferred into the
        # main loop so it doesn't contend with startup DMA

        # ---------------- fused projections + RoPE + attention ----------------
        with ExitStack() as p2:
            xpool = p2.enter_context(tc.tile_pool(name="xpool", bufs=2))
            rpool = p2.enter_context(tc.tile_pool(name="rope", bufs=3))
            wpool = p2.enter_context(tc.tile_pool(name="wpool", bufs=4))
            apool = p2.enter_context(tc.tile_pool(name="acc", bufs=2))
            npool = p2.enter_context(tc.tile_pool(name="norm", bufs=2))
            copool = p2.enter_context(tc.tile_pool(name="cout", bufs=2))
            # PSUM: 8 banks of [128, 2KB]: pq=1, pk=1, pvt=1, ps=2+pd=1, pc=2
            pq_pool = p2.enter_context(tc.tile_pool(name="pq", bufs=1, space="PSUM"))
            pk_pool = p2.enter_context(tc.tile_pool(name="pk", bufs=1, space="PSUM"))
            pvt_pool = p2.enter_context(tc.tile_pool(name="pvt", bufs=1, space="PSUM"))
            ps_pool = p2.enter_context(tc.tile_pool(name="ps", bufs=2, space="PSUM"))
            pc_pool = p2.enter_context(tc.tile_pool(name="pc", bufs=2, space="PSUM"))

            def load_xt(qc):
                t = xpool.tile([128, NKIN, QC], F16, tag="xt")
                nc.sync.dma_start(
                    out=t,
                    in_=xT.rearrange("(t p) m -> p t m", p=128)
                          [:, :, qc * QC:(qc + 1) * QC])
                return t

            xt_next = load_xt(0)
            nc.sync.dma_start(out=wq_sb,
                              in_=wqT.rearrange("(t p) m -> p t m", p=128))
            nc.sync.dma_start(out=wk_sb,
                              in_=wkT.rearrange("(t p) m -> p t m", p=128))
            nc.sync.dma_start(out=wv_sb,
                              in_=wvT.rearrange("(t p) m -> p t m", p=128))
            nc.sync.dma_start(out=cos_sb, in_=cosT[:, :])
            nc.sync.dma_start(out=sin_sb, in_=sinT[:, :])
            for qc in range(NQ):
                q0 = qc * QC
                # -------- projections --------
                # Q heads projected sequentially into a single PSUM bank;
                # K/V interleaved in their own banks.
                xt = xt_next
                q2 = rpool.tile([128, HPC, QC], F16, tag="q2")
                for h in range(HPC):
                    pq = pq_pool.tile([128, QC], F32, tag="pq")
                    for ki in range(NKIN):
                        nc.tensor.matmul(
                            pq, lhsT=wq_sb[:, ki, h * D:(h + 1) * D],
                            rhs=xt[:, ki, :], start=(ki == 0),
                            stop=(ki == NKIN - 1))
                    nc.vector.tensor_copy(q2[:, h, :], pq)
                pk = pk_pool.tile([128, QC], F32, tag="pk")
                pvt = pvt_pool.tile([128, QC], F32, tag="pvt")
                for ki in range(NKIN):
                    st = ki == 0
                    sp = ki == NKIN - 1
                    nc.tensor.matmul(pk, lhsT=wk_sb[:, ki, :], rhs=xt[:, ki, :],
                                     start=st, stop=sp)
                    nc.tensor.matmul(pvt, lhsT=wv_sb[:, ki, :], rhs=xt[:, ki, :],
                                     start=st, stop=sp)
                # prefetch next x-chunk while this chunk's attention runs
                if qc + 1 < NQ:
                    xt_next = load_xt(qc + 1)
                if qc == 1:
                    # deferred 8MB wo load, streams under the remaining loop
                    nc.scalar.dma_start(
                        out=wo_sb,
                        in_=woT.rearrange("(t p) m -> p t m", p=128))
                # VT -> V via DMA XBAR transpose (fp16, runs on DMA engines:
                # out[p, m, d] = in[d, m*128 + p], exactly vt's layout)
                vtT = rpool.tile([128, QC], F16, tag="vtT")
                nc.vector.tensor_copy(vtT, pvt)
                nc.sync.dma_start_transpose(
                    out=vt[:, qc * 4:(qc + 1) * 4, :], in_=vtT)

                # -------- RoPE (all-fp16 on DVE) --------
                cos_c = cos_sb[:, q0:q0 + QC]
                sin_c = sin_sb[:, q0:q0 + QC]
                k1 = rpool.tile([128, QC], F16, tag="k1")
                nc.vector.tensor_copy(k1, pk)

                def rope(dst, src):
                    rot = rpool.tile([128, QC], F16, tag="rot")
                    nc.vector.tensor_scalar_mul(rot[0:64, :], src[64:128, :], -1.0)
                    nc.vector.tensor_copy(rot[64:128, :], src[0:64, :])
                    nc.vector.tensor_mul(dst, src, cos_c)
                    nc.vector.tensor_mul(rot, rot, sin_c)
                    nc.vector.tensor_add(dst, dst, rot)

                for h in range(HPC):
                    rope(qt[:, h, q0:q0 + QC], q2[:, h, :])
                rope(kt[:, q0:q0 + QC], k1)

                # -------- attention for this q-chunk --------
                # both heads share one software-pipelined stream: scores run
                # one k-tile ahead of the PV matmuls so the PE never waits on
                # the ACT exp latency
                nk = (qc + 1) * 4
                LAG = 1
                pcs = [pc_pool.tile([128, QC], F32, tag="pc", name=f"pc{h}")
                       for h in range(HPC)]
                accs = [apool.tile([128, QC], F16, tag="acc", name=f"acc{h}")
                        for h in range(HPC)]
                wts = {}

                def emit_scores(h, ki):
                    k0 = ki * KT
                    ps = ps_pool.tile([128, QC], F32, tag="ps")
                    nc.tensor.matmul(ps, lhsT=kt[:, k0:k0 + KT],
                                     rhs=qt[:, h, q0:q0 + QC],
                                     start=True, stop=True)
                    wt = wpool.tile([128, QC], F16, tag="wt")
                    nc.scalar.activation(wt, ps,
                                         mybir.ActivationFunctionType.Exp,
                                         scale=INV_SQRT_D, bias=ebias)
                    if k0 + KT - 1 > q0:
                        # keep where (q0+j) - (k0+p) >= 0
                        nc.gpsimd.affine_select(
                            out=wt, in_=wt, pattern=[[1, QC]],
                            compare_op=mybir.AluOpType.is_ge, fill=0.0,
                            base=q0 - k0, channel_multiplier=-1)
                    wts[(h, ki)] = wt

                def emit_pv(h, ki):
                    wt = wts.pop((h, ki))
                    nc.tensor.matmul(pcs[h], lhsT=vt[:, ki, :], rhs=wt,
                                     start=(ki == 0), stop=(ki == nk - 1))
                    if ki == 0:
                        nc.vector.tensor_copy(accs[h], wt)
                    else:
                        nc.vector.tensor_add(accs[h], accs[h], wt)

                for ki in range(nk):
                    for h in range(HPC):
                        emit_scores(h, ki)
                    if ki >= LAG:
                        for h in range(HPC):
                            emit_pv(h, ki - LAG)
                for ki in range(nk - LAG, nk):
                    for h in range(HPC):
                        emit_pv(h, ki)
                for h in range(HPC):
                    # denominator: all-ones matmul reduces over partitions AND
                    # broadcasts the sum to every partition in one instruction
                    pd = ps_pool.tile([128, QC], F32, tag="pd", bufs=1)
                    nc.tensor.matmul(pd, lhsT=ones128, rhs=accs[h],
                                     start=True, stop=True)
                    # 1/d as exp(-ln(d)) on ACT (DVE reciprocal costs 3.4us
                    # per tile; custom-DVE approx ops don't compile here)
                    lg = npool.tile([128, QC], F32, tag="lg")
                    nc.scalar.activation(lg, pd,
                                         mybir.ActivationFunctionType.Ln,
                                         scale=1.0)
                    rec = npool.tile([128, QC], F16, tag="rec")
                    nc.scalar.activation(rec, lg,
                                         mybir.ActivationFunctionType.Exp,
                                         scale=-1.0)
                    cout = copool.tile([128, QC], F16, tag="cout")
                    nc.vector.tensor_mul(cout, pcs[h], rec)
                    nc.gpsimd.dma_start(
                        out=cc_inA[qc, h * D:(h + 1) * D, :],
                        in_=cout[:, 0:CH])
                    nc.gpsimd.dma_start(
                        out=cc_inB[qc, h * D:(h + 1) * D, :],
                        in_=cout[:, CH:QC])

        # ---------------- all-to-all (seq-shard the context) ----------------
        # two column-half AllToAlls; out-proj of half A overlaps the second
        # collective. No barriers: tile deps order the cc_in writes, the
        # collectives, and the cc_out reads.
        nc.gpsimd.collective_compute(
            "AllToAll",
            mybir.AluOpType.bypass,
            replica_groups=[list(range(NCORES))],
            ins=[cc_inA[:, :, :]],
            outs=[cc_outA[:, :, :]],
        )
        nc.gpsimd.collective_compute(
            "AllToAll",
            mybir.AluOpType.bypass,
            replica_groups=[list(range(NCORES))],
            ins=[cc_inB[:, :, :]],
            outs=[cc_outB[:, :, :]],
        )

        # ---------------- out-proj for my 512 columns ----------------
        with ExitStack() as p3:
            cpool = p3.enter_context(tc.tile_pool(name="cpool", bufs=1))
            opool = p3.enter_context(tc.tile_pool(name="opool", bufs=2))
            po_pool = p3.enter_context(tc.tile_pool(name="po", bufs=2, space="PSUM"))
            for half, cc_out_h in ((0, cc_outA), (1, cc_outB)):
                cc_sb = cpool.tile([128, NKIN, CH], F16, tag=f"cc_sb{half}",
                                   name=f"cc_sb{half}")
                nc.sync.dma_start(
                    out=cc_sb,
                    in_=cc_out_h.rearrange("c (t p) m -> p (c t) m", p=128))
                for dt in range(NKIN):
                    po = po_pool.tile([128, CH], F32, tag="po", name="po")
                    for ki in range(NKIN):
                        nc.tensor.matmul(
                            po, lhsT=wo_sb[:, ki, dt * 128:(dt + 1) * 128],
                            rhs=cc_sb[:, ki, :], start=(ki == 0),
                            stop=(ki == NKIN - 1))
                    ot = opool.tile([128, CH], F32, tag="ot", name="ot")
                    nc.vector.tensor_copy(ot, po)
                    nc.sync.dma_start(
                        out=outT[dt * 128:(dt + 1) * 128,
                                 half * CH:(half + 1) * CH], in_=ot)

    _split_sync_waits(nc)
    return nc


_NC_CACHE = None


def _get_nc():
    global _NC_CACHE
    if _NC_CACHE is None:
        _NC_CACHE = build_nc()
    return _NC_CACHE


def _make_in_maps(x, cos, sin, Wq, Wk, Wv, Wo):
    xT = np.ascontiguousarray(x.reshape(S, DIN).T.astype(np.float16))
    cosT = np.ascontiguousarray(cos.T.astype(np.float16))
    sinT = np.ascontiguousarray(sin.T.astype(np.float16))
    woT = np.ascontiguousarray(Wo.T.astype(np.float16))
    in_maps = []
    for c in range(NCORES):
        g = c // 2
        in_maps.append({
            "xT": xT,
            "wqT": np.ascontiguousarray(Wq[c * 256:(c + 1) * 256, :].T
                                        .astype(np.float16)),
            "wkT": np.ascontiguousarray(Wk[g * 128:(g + 1) * 128, :].T
                                        .astype(np.float16)),
            "wvT": np.ascontiguousarray(Wv[g * 128:(g + 1) * 128, :].T
                                        .astype(np.float16)),
            "cosT": cosT,
            "sinT": sinT,
            "woT": woT,
        })
    return in_maps


def run(x, cos, sin, Wq, Wk, Wv, Wo, trace=False, tmpdir=None):
    nc = _get_nc()
    in_maps = _make_in_maps(x, cos, sin, Wq, Wk, Wv, Wo)
    res = run_bass_kernel_spmd(nc, in_maps, list(range(NCORES)), trace=trace,
                               tmpdir=tmpdir)
    out = np.empty((1, S, DIN), dtype=np.float32)
    for c in range(NCORES):
        out[0, c * QC:(c + 1) * QC, :] = res.results[c]["outT"].T
    return out, res


def kernel(x, mask, cos, sin, Wq, Wk, Wv, Wo):
    out, _ = run(np.asarray(x, dtype=np.float32), np.asarray(cos, np.float32),
                 np.asarray(sin, np.float32), np.asarray(Wq, np.float32),
                 np.asarray(Wk, np.float32), np.asarray(Wv, np.float32),
                 np.asarray(Wo, np.float32))
    return out


# revision 34
# speedup vs baseline: 3.7546x; 1.0260x over previous
"""GQA kernel for Trainium2, 8 NeuronCores.

Problem: x[1,4096,2048], H=16 heads, G=4 kv-groups, D=128, causal mask,
RoPE on q/k, out-proj. Sharding: 2 heads per core (core c -> heads 2c,2c+1,
kv-group c//2); out-proj sharded by output feature rows (core c -> dout
rows c*256..c*256+255, all 4096 positions).

Pipeline (all fp16 data path, fp32 PSUM accumulation):
  per q-chunk of 512:  project Q/K/V (K and V share one PSUM bank
  sequentially), RoPE on DVE, causal attention with scores one k-tile ahead
  of the PV matmuls (ACT exp latency hidden), softmax denominator via
  all-ones matmul (partition reduce+broadcast in one instruction) and
  1/d = exp(-ln d) on ACT. Context rows go out via a per-chunk AllGather
  that runs on the CC stream while compute continues; the out-projection
  for chunk qc-3 is interleaved into iteration qc, so phase 3 has no
  serial tail beyond the last chunk's gather.

Host supplies partition-major pre-layouts so every DMA moves >=8KB
contiguous per partition.
"""

import sys

for _p in ("/opt/trn_rl_repo",):
    if _p not in sys.path:
        sys.path.append(_p)

from contextlib import ExitStack

import numpy as np

import concourse.bass as bass
import concourse.tile as tile
from concourse import mybir
from concourse.bass_utils import run_bass_kernel_spmd

F32 = mybir.dt.float32
F16 = mybir.dt.float16
S = 4096
MAX_WAITS = 1  # walrus CoreV3 rejects instructions with more sync waits


def _split_sync_waits(nc, maxw=MAX_WAITS):
    """Hoist excess sem waits onto NOPs inserted before the instruction on
    the same engine queue (queue order makes this equivalent)."""
    from concourse import mybir as mb
    n = 0
    for bassbb in nc.bb_map.values():
        bb = bassbb.bb
        insts = list(bb.instructions)
        out = []
        changed = False
        for ins in insts:
            si = ins.sync_info
            if si is not None and si.on_wait and len(si.on_wait) > maxw:
                waits = list(si.on_wait)
                head, rest = waits[:-maxw], waits[-maxw:]
                while head:
                    chunk, head = head[:maxw], head[maxw:]
                    n += 1
                    nop = mb.InstNoOp(
                        name=f"I-ws{n}",
                        engine=ins.engine,
                        ins=[],
                        outs=[],
                        sync_info=mb.SyncInfo(on_wait=chunk, on_update=[]),
                    )
                    nc.register_instruction(nop)
                    out.append(nop)
                ins.sync_info = mb.SyncInfo(
                    on_wait=rest, on_update=list(si.on_update or []))
                changed = True
            out.append(ins)
        if changed:
            try:
                bb.instructions[:] = out
            except TypeError:
                bb.set_instructions(out)
    return n


DIN = 2048
D = 128
HPC = 2          # heads per core
DOUT_PC = HPC * D  # out-proj rows per core
NCORES = 8
QC = 512         # q-chunk (free dim per matmul)
NQ = S // QC     # 8 q-chunks
KT = 128         # k tile (partition dim)
NKIN = DIN // 128  # 16 contraction tiles for projections
NDT = DOUT_PC // 128  # 2 dout tiles per core
INV_SQRT_D = 1.0 / np.sqrt(D)
EXP_BIAS = -2.0  # keeps fp16 softmax sums well inside range; cancels in norm
OP_LAG = 3       # out-proj trails attention by this many chunks


def build_nc():
    nc = bass.Bass(num_devices=NCORES)

    # partition-major pre-layouts (big contiguous runs per partition)
    xP = nc.dram_tensor("xP", [128, NQ, NKIN, QC], F16, kind="ExternalInput")
    wqP = nc.dram_tensor("wqP", [128, NKIN, HPC * D], F16, kind="ExternalInput")
    wkP = nc.dram_tensor("wkP", [128, NKIN, D], F16, kind="ExternalInput")
    wvP = nc.dram_tensor("wvP", [128, NKIN, D], F16, kind="ExternalInput")
    woP = nc.dram_tensor("woP", [128, NKIN, DOUT_PC], F16, kind="ExternalInput")
    cosT = nc.dram_tensor("cosT", [D, S], F16, kind="ExternalInput")
    sinT = nc.dram_tensor("sinT", [D, S], F16, kind="ExternalInput")
    outP = nc.dram_tensor("outP", [128, NDT, S], F32, kind="ExternalOutput")

    # exchange buffers (collectives can't touch I/O tensors)
    cc_in = nc.dram_tensor("cc_in", [NQ, HPC * D, QC], F16)
    cc_all = nc.dram_tensor("cc_all", [NQ, NCORES, HPC * D, QC], F16)

    with ExitStack() as ctx:
        tc = ctx.enter_context(tile.TileContext(nc))

        res = ctx.enter_context(tc.tile_pool(name="res", bufs=1))
        # resident SBUF tensors
        qt = res.tile([128, HPC, S], F16, tag="qt")          # QT per head
        kt = res.tile([128, S], F16, tag="kt")               # KT (shared group)
        vt = res.tile([128, S // 128, D], F16, tag="vt")     # V as s-tiles
        wq_sb = res.tile([128, NKIN, HPC * D], F16, tag="wq")
        wk_sb = res.tile([128, NKIN, D], F16, tag="wk")
        wv_sb = res.tile([128, NKIN, D], F16, tag="wv")
        wo_sb = res.tile([128, NKIN, DOUT_PC], F16, tag="wo")
        cos_sb = res.tile([128, S], F16, tag="cos")
        sin_sb = res.tile([128, S], F16, tag="sin")
        ebias = res.tile([128, 1], F32, tag="ebias")         # exp bias const
        ones128 = res.tile([128, 128], F16, tag="ones128")   # partition reduce+bcast

        nc.vector.memset(ebias, EXP_BIAS)
        nc.vector.memset(ones128, 1.0)

        nc.sync.dma_start(out=wq_sb, in_=wqP[:, :, :])
        nc.sync.dma_start(out=wk_sb, in_=wkP[:, :, :])
        nc.sync.dma_start(out=wv_sb, in_=wvP[:, :, :])
        nc.sync.dma_start(out=wo_sb, in_=woP[:, :, :])
        nc.sync.dma_start(out=cos_sb, in_=cosT[:, :])
        nc.sync.dma_start(out=sin_sb, in_=sinT[:, :])

        # ---------------- fused main loop ----------------
        with ExitStack() as p2:
            xpool = p2.enter_context(tc.tile_pool(name="xpool", bufs=2))
            rpool = p2.enter_context(tc.tile_pool(name="rope", bufs=3))
            wpool = p2.enter_context(tc.tile_pool(name="wpool", bufs=6))
            apool = p2.enter_context(tc.tile_pool(name="acc", bufs=2))
            npool = p2.enter_context(tc.tile_pool(name="norm", bufs=2))
            copool = p2.enter_context(tc.tile_pool(name="cout", bufs=2))
            cpool = p2.enter_context(tc.tile_pool(name="cpool", bufs=2))
            opool = p2.enter_context(tc.tile_pool(name="opool", bufs=2))
            # PSUM: 8 banks of [128, 2KB]:
            #   pq/po shared tag (2) + pkv (1) + ps (2) + pd (1) + pc (2)
            pq_pool = p2.enter_context(tc.tile_pool(name="pq", bufs=2, space="PSUM"))
            pkv_pool = p2.enter_context(tc.tile_pool(name="pkv", bufs=1, space="PSUM"))
            ps_pool = p2.enter_context(tc.tile_pool(name="ps", bufs=2, space="PSUM"))
            pc_pool = p2.enter_context(tc.tile_pool(name="pc", bufs=2, space="PSUM"))

            def load_xt(qc):
                t = xpool.tile([128, NKIN, QC], F16, tag="xt")
                nc.sync.dma_start(out=t, in_=xP[:, qc, :, :])
                return t

            def emit_outproj(qc):
                """out-proj of chunk qc (gathered OP_LAG chunks ago)."""
                cc_sb = cpool.tile([128, NKIN, QC], F16, tag="cc_sb",
                                   name="cc_sb")
                nc.sync.dma_start(
                    out=cc_sb,
                    in_=cc_all[qc].rearrange("c (t p) m -> p (c t) m", p=128))
                for dt in range(NDT):
                    po = pq_pool.tile([128, QC], F32, tag="pq", name="po")
                    for ki in range(NKIN):
                        nc.tensor.matmul(
                            po, lhsT=wo_sb[:, ki, dt * 128:(dt + 1) * 128],
                            rhs=cc_sb[:, ki, :], start=(ki == 0),
                            stop=(ki == NKIN - 1))
                    ot = opool.tile([128, QC], F32, tag="ot", name="ot")
                    nc.scalar.copy(ot, po)
                    nc.sync.dma_start(
                        out=outP[:, dt, qc * QC:(qc + 1) * QC], in_=ot)

            xt_next = load_xt(0)
            for qc in range(NQ):
                q0 = qc * QC
                # -------- projections --------
                xt = xt_next
                q2 = rpool.tile([128, HPC, QC], F16, tag="q2")
                for h in range(HPC):
                    pq = pq_pool.tile([128, QC], F32, tag="pq")
                    for ki in range(NKIN):
                        nc.tensor.matmul(
                            pq, lhsT=wq_sb[:, ki, h * D:(h + 1) * D],
                            rhs=xt[:, ki, :], start=(ki == 0),
                            stop=(ki == NKIN - 1))
                    nc.vector.tensor_copy(q2[:, h, :], pq)
                # K then V through one PSUM bank (freed by the k1 copy)
                pk = pkv_pool.tile([128, QC], F32, tag="pkv", name="pk")
                for ki in range(NKIN):
                    nc.tensor.matmul(pk, lhsT=wk_sb[:, ki, :], rhs=xt[:, ki, :],
                                     start=(ki == 0), stop=(ki == NKIN - 1))
                k1 = rpool.tile([128, QC], F16, tag="k1")
                nc.vector.tensor_copy(k1, pk)
                pvt = pkv_pool.tile([128, QC], F32, tag="pkv", name="pvt")
                for ki in range(NKIN):
                    nc.tensor.matmul(pvt, lhsT=wv_sb[:, ki, :], rhs=xt[:, ki, :],
                                     start=(ki == 0), stop=(ki == NKIN - 1))
                # prefetch next x-chunk while this chunk's attention runs
                if qc + 1 < NQ:
                    xt_next = load_xt(qc + 1)

                # VT -> V via DMA XBAR transpose (fp16, runs on DMA engines:
                # out[p, m, d] = in[d, m*128 + p], exactly vt's layout)
                vtT = rpool.tile([128, QC], F16, tag="vtT")
                nc.vector.tensor_copy(vtT, pvt)
                nc.sync.dma_start_transpose(
                    out=vt[:, qc * 4:(qc + 1) * 4, :], in_=vtT)

                # -------- RoPE (all-fp16 on DVE) --------
                cos_c = cos_sb[:, q0:q0 + QC]
                sin_c = sin_sb[:, q0:q0 + QC]

                def rope(dst, src):
                    rot = rpool.tile([128, QC], F16, tag="rot")
                    nc.vector.tensor_scalar_mul(rot[0:64, :], src[64:128, :], -1.0)
                    nc.vector.tensor_copy(rot[64:128, :], src[0:64, :])
                    nc.vector.tensor_mul(dst, src, cos_c)
                    nc.vector.tensor_mul(rot, rot, sin_c)
                    nc.vector.tensor_add(dst, dst, rot)

                for h in range(HPC):
                    rope(qt[:, h, q0:q0 + QC], q2[:, h, :])
                rope(kt[:, q0:q0 + QC], k1)

                # -------- attention for this q-chunk --------
                nk = (qc + 1) * 4
                LAG = 1
                pcs = [pc_pool.tile([128, QC], F32, tag="pc", name=f"pc{h}")
                       for h in range(HPC)]
                accs = [apool.tile([128, QC], F16, tag="acc", name=f"acc{h}")
                        for h in range(HPC)]
                wts = {}

                def emit_scores(h, ki):
                    k0 = ki * KT
                    ps = ps_pool.tile([128, QC], F32, tag="ps")
                    nc.tensor.matmul(ps, lhsT=kt[:, k0:k0 + KT],
                                     rhs=qt[:, h, q0:q0 + QC],
                                     start=True, stop=True)
                    wt = wpool.tile([128, QC], F16, tag="wt")
                    nc.scalar.activation(wt, ps,
                                         mybir.ActivationFunctionType.Exp,
                                         scale=INV_SQRT_D, bias=ebias)
                    if k0 + KT - 1 > q0:
                        # keep where (q0+j) - (k0+p) >= 0
                        nc.gpsimd.affine_select(
                            out=wt, in_=wt, pattern=[[1, QC]],
                            compare_op=mybir.AluOpType.is_ge, fill=0.0,
                            base=q0 - k0, channel_multiplier=-1)
                    wts[(h, ki)] = wt

                def emit_pv(h, ki):
                    wt = wts.pop((h, ki))
                    nc.tensor.matmul(pcs[h], lhsT=vt[:, ki, :], rhs=wt,
                                     start=(ki == 0), stop=(ki == nk - 1))
                    if ki == 0:
                        nc.vector.tensor_copy(accs[h], wt)
                    else:
                        nc.vector.tensor_add(accs[h], accs[h], wt)

                for ki in range(nk):
                    for h in range(HPC):
                        emit_scores(h, ki)
                    if ki >= LAG:
                        for h in range(HPC):
                            emit_pv(h, ki - LAG)
                for ki in range(nk - LAG, nk):
                    for h in range(HPC):
                        emit_pv(h, ki)
                for h in range(HPC):
                    # denominator: all-ones matmul reduces over partitions AND
                    # broadcasts the sum to every partition in one instruction
                    pd = ps_pool.tile([128, QC], F32, tag="pd", bufs=1)
                    nc.tensor.matmul(pd, lhsT=ones128, rhs=accs[h],
                                     start=True, stop=True)
                    # 1/d as exp(-ln(d)) on ACT
                    lg = npool.tile([128, QC], F32, tag="lg")
                    nc.scalar.activation(lg, pd,
                                         mybir.ActivationFunctionType.Ln,
                                         scale=1.0)
                    rec = npool.tile([128, QC], F16, tag="rec")
                    nc.scalar.activation(rec, lg,
                                         mybir.ActivationFunctionType.Exp,
                                         scale=-1.0)
                    cout = copool.tile([128, QC], F16, tag="cout")
                    nc.vector.tensor_mul(cout, pcs[h], rec)
                    nc.gpsimd.dma_start(
                        out=cc_in[qc, h * D:(h + 1) * D, :], in_=cout)

                # gather this chunk's context rows from all cores (runs on
                # the CC stream while the next chunks compute)
                nc.gpsimd.collective_compute(
                    "AllGather",
                    mybir.AluOpType.bypass,
                    replica_groups=[list(range(NCORES))],
                    ins=[cc_in[qc]],
                    outs=[cc_all[qc]],
                )

                # out-proj of the chunk gathered OP_LAG iterations ago
                if qc >= OP_LAG:
                    emit_outproj(qc - OP_LAG)

            for qc in range(NQ - OP_LAG, NQ):
                emit_outproj(qc)

    _split_sync_waits(nc)
    return nc


_NC_CACHE = None


def _get_nc():
    global _NC_CACHE
    if _NC_CACHE is None:
        _NC_CACHE = build_nc()
    return _NC_CACHE


def _pmajor(a2d):
    """[T*128, M] -> [128, T, M] with row t*128+p landing at [p, t]."""
    t = a2d.shape[0] // 128
    return np.ascontiguousarray(
        a2d.reshape(t, 128, a2d.shape[1]).transpose(1, 0, 2))


def _make_in_maps(x, cos, sin, Wq, Wk, Wv, Wo):
    xT = x.reshape(S, DIN).T.astype(np.float16)          # [DIN, S]
    xPm = _pmajor(xT)                                    # [128, 16, 4096]
    xP = np.ascontiguousarray(
        xPm.reshape(128, NKIN, NQ, QC).transpose(0, 2, 1, 3))
    cosT = np.ascontiguousarray(cos.T.astype(np.float16))
    sinT = np.ascontiguousarray(sin.T.astype(np.float16))
    in_maps = []
    for c in range(NCORES):
        g = c // 2
        in_maps.append({
            "xP": xP,
            "wqP": _pmajor(Wq[c * 256:(c + 1) * 256, :].T.astype(np.float16)),
            "wkP": _pmajor(Wk[g * 128:(g + 1) * 128, :].T.astype(np.float16)),
            "wvP": _pmajor(Wv[g * 128:(g + 1) * 128, :].T.astype(np.float16)),
            "woP": _pmajor(np.ascontiguousarray(
                Wo[c * 256:(c + 1) * 256, :]).T.astype(np.float16)),
            "cosT": cosT,
            "sinT": sinT,
        })
    return in_maps


def run(x, cos, sin, Wq, Wk, Wv, Wo, trace=False, tmpdir=None):
    nc = _get_nc()
    in_maps = _make_in_maps(x, cos, sin, Wq, Wk, Wv, Wo)
    res = run_bass_kernel_spmd(nc, in_maps, list(range(NCORES)), trace=trace,
                               tmpdir=tmpdir)
    out = np.empty((1, S, DIN), dtype=np.float32)
    for c in range(NCORES):
        op = res.results[c]["outP"]                      # [128, NDT, S]
        for dt in range(NDT):
            out[0, :, c * 256 + dt * 128:c * 256 + (dt + 1) * 128] = op[:, dt, :].T
    return out, res


def kernel(x, mask, cos, sin, Wq, Wk, Wv, Wo):
    out, _ = run(np.asarray(x, dtype=np.float32), np.asarray(cos, np.float32),
                 np.asarray(sin, np.float32), np.asarray(Wq, np.float32),
                 np.asarray(Wk, np.float32), np.asarray(Wv, np.float32),
                 np.asarray(Wo, np.float32))
    return out
